# revision 24
# baseline (speedup 1.0000x reference)
# MixGAT layer (GATConv + beta-mix swish) on 8 Trainium2 NeuronCores.
#
# Strategy (dst-node sharding): nodes are packed into fixed 32-dst "windows"
# (2D bin-packing by lo/hi in-degree so every window fits a static block
# budget); 200 windows per core.  Per superblock (4 windows = 128 dsts):
#   - dma_gather pulls each edge's RAW x row (fp16, 256B) from a host-cast
#     table; aggregation of raw x is exchanged with the W projection
#     (out = (sum_e w_e x[src]) @ W per head).
#   - alpha = lrelu(a_src[src]+a_dst[dst]) -> exp on ACT; a_src/a_dst are
#     per-node values computed on device in launch 1 and expanded per-edge
#     on host (indexing only).
#   - mw[e,(h,c)] = onehot(dst slot) * expa[e,h]; one matmul per 128-edge
#     block accumulates U[(h,c), f] = sum_e mw * x[src]; a second tiny
#     matmul accumulates the softmax denominators dn[c,h].
#   - U is transposed on the PE and projected through W per head, giving
#     node-major z rows; swish postproc and one contiguous output DMA per
#     128 dsts.
#
# kernel(**inputs) is self-contained: preprocessing is pure numpy indexing,
# the device kernels run via run_bass_kernel_spmd on cores 0-7.

import os

import numpy as np

import concourse.bass as bass
import concourse.mybir as mybir
import concourse.tile as tile
from concourse import bacc
from concourse.bass_utils import run_bass_kernel_spmd

F32 = mybir.dt.float32
F16 = mybir.dt.float16
I16 = mybir.dt.int16

# problem constants
N_NODES = 50000
IN_DIM = 128
HEADS = 4
OUT_DIM = 32
LEAKY_SLOPE = 0.2
BETA = 0.5
CMIX = 1.2
N_CORES = 8

# static schedule constants
WIN = 32          # dsts per window / group
BLK = 128         # edges per block (matmul contraction)
WPC = 200         # windows per core
NSB = WPC // 4    # superblocks (128 dsts) per core
NPAIR = NSB // 2  # gather pairs (2 superblocks per gather)
SPLIT = 32768     # int16-addressable table split
NPAD = 50176      # padded table rows (multiple of 128)
DEAD = 100.0      # colidx value for dead slots


class Cfg:
    def __init__(self, lob, hib, bias_nonzero=False, n_cores=N_CORES):
        self.lob = lob                  # lo blocks per window
        self.hib = hib                  # hi blocks per window
        self.nblk = 4 * (lob + hib)     # block columns per superblock
        self.nlo = 4 * lob * BLK        # lo slots per superblock
        self.nhi = 4 * hib * BLK
        self.bias_nonzero = bias_nonzero
        self.n_cores = n_cores


# ---------------------------------------------------------------- host side

def _wrap16(v):
    """idx vector [S*16] -> dma_gather idx layout [128, S] int16."""
    s = v.reshape(-1, 16).T                      # [16, S]
    return np.tile(s, (8, 1)).astype(np.int16)   # [128, S]


def assign_windows(deg_lo, deg_hi, lo_cap, hi_cap, n_win):
    """Snake-deal nodes to windows by hi-degree, then repair cap violations.
    Returns win_of[n]. Raises if infeasible under the caps."""
    n = len(deg_lo)
    order = np.argsort(-deg_hi, kind="stable")
    idx = np.arange(n)
    row = idx // n_win
    k = idx % n_win
    w = np.where(row % 2 == 0, k, n_win - 1 - k)
    win_of = np.empty(n, dtype=np.int64)
    win_of[order] = w

    cnt = np.bincount(win_of, minlength=n_win)
    assert cnt.max() <= WIN
    # fakes for empty slots count against lo capacity
    losum = np.bincount(win_of, weights=deg_lo, minlength=n_win) + (WIN - cnt)
    hisum = np.bincount(win_of, weights=deg_hi, minlength=n_win)

    # windows -> node lists for swapping
    by_win = [[] for _ in range(n_win)]
    for node in range(n):
        by_win[win_of[node]].append(node)

    def try_fix(sums, caps, other, ocap, deg, odeg):
        for _ in range(20000):
            wbad = int(np.argmax(sums))
            if sums[wbad] <= caps:
                return True
            # heaviest node in the bad window
            nodes_bad = by_win[wbad]
            a = max(nodes_bad, key=lambda x: deg[x])
            done = False
            for wgood in np.argsort(sums)[:64]:
                wgood = int(wgood)
                if wgood == wbad:
                    continue
                for b in sorted(by_win[wgood], key=lambda x: deg[x])[:8]:
                    d_s = deg[a] - deg[b]
                    d_o = odeg[a] - odeg[b]
                    if (sums[wbad] - d_s <= caps and sums[wgood] + d_s <= caps
                            and other[wbad] - d_o <= ocap
                            and other[wgood] + d_o <= ocap):
                        by_win[wbad].remove(a)
                        by_win[wgood].remove(b)
                        by_win[wbad].append(b)
                        by_win[wgood].append(a)
                        win_of[a], win_of[b] = wgood, wbad
                        sums[wbad] -= d_s
                        sums[wgood] += d_s
                        other[wbad] -= d_o
                        other[wgood] += d_o
                        done = True
                        break
                if done:
                    break
            if not done:
                return False
        return False

    if not try_fix(hisum, hi_cap, losum, lo_cap, deg_hi, deg_lo):
        raise RuntimeError("hi repair failed")
    if not try_fix(losum, lo_cap, hisum, hi_cap, deg_lo, deg_hi):
        raise RuntimeError("lo repair failed")
    assert losum.max() <= lo_cap and hisum.max() <= hi_cap
    return win_of


def preprocess(edge_index, n_cores=N_CORES):
    """Window assignment + static per-core gather/colidx/edge-stream arrays.

    Returns (cfg, per_core list, new_row[n] output permutation,
    sn/dn index arrays for the asd expansion)."""
    src0 = np.asarray(edge_index[0], dtype=np.int64)
    dst0 = np.asarray(edge_index[1], dtype=np.int64)
    loop = np.arange(N_NODES, dtype=np.int64)
    src = np.concatenate([src0, loop])
    dst = np.concatenate([dst0, loop])
    lo_mask_e = src < SPLIT
    deg_lo = np.bincount(dst[lo_mask_e], minlength=N_NODES)
    deg_hi = np.bincount(dst[~lo_mask_e], minlength=N_NODES)

    n_win = n_cores * WPC
    cfg = None
    win_of = None
    for lob, hib in ((6, 3), (7, 3), (7, 4), (8, 4)):
        try:
            win_of = assign_windows(deg_lo, deg_hi, lob * BLK, hib * BLK, n_win)
            cfg = Cfg(lob, hib)
            break
        except RuntimeError:
            continue
    assert cfg is not None, "window packing failed at all cap levels"
    lob, hib = cfg.lob, cfg.hib

    # slot assignment within windows (order of appearance)
    order = np.argsort(win_of, kind="stable")
    slot = np.empty(N_NODES, dtype=np.int64)
    bounds = np.searchsorted(win_of[order], np.arange(n_win + 1))
    for w in range(n_win):
        seg = order[bounds[w]:bounds[w + 1]]
        slot[seg] = np.arange(len(seg))
    # output row of each original node; the per-pair output DMA interleaves
    # the two superblocks of a pair: row = 256*pair + 2*(g*32+slot) + s
    core_of = win_of // WPC
    wl = win_of % WPC
    sb_of, g_of = wl // 4, wl % 4
    new_row = (core_of * (WPC * WIN) + (sb_of // 2) * 256
               + 2 * (g_of * WIN + slot) + (sb_of % 2))

    # per-edge: window / slot of the dst
    e_win = win_of[dst]
    e_slot = slot[dst]
    e_order = np.argsort(e_win, kind="stable")
    eb = np.searchsorted(e_win[e_order], np.arange(n_win + 1))

    per_core = []
    for c in range(n_cores):
        ilo = np.zeros((NSB, 4, lob, BLK), dtype=np.int64)
        ihi = np.zeros((NSB, 4, hib, BLK), dtype=np.int64)
        clo = np.full((NSB, 4, lob, BLK), DEAD, dtype=np.float16)
        chi = np.full((NSB, 4, hib, BLK), DEAD, dtype=np.float16)
        snlo = np.zeros((NSB, 4, lob, BLK), dtype=np.int64)
        dnlo = np.zeros((NSB, 4, lob, BLK), dtype=np.int64)
        snhi = np.zeros((NSB, 4, hib, BLK), dtype=np.int64)
        dnhi = np.zeros((NSB, 4, hib, BLK), dtype=np.int64)
        for wl in range(WPC):
            w = c * WPC + wl
            sb, g = wl // 4, wl % 4
            seg = e_order[eb[w]:eb[w + 1]]
            es, ec, ed = src[seg], e_slot[seg], dst[seg]
            m = es < SPLIT
            # lo side, with fake self-edges for empty slots
            nfake = WIN - (bounds[w + 1] - bounds[w])
            ls = np.concatenate([es[m], np.zeros(nfake, dtype=np.int64)])
            lc = np.concatenate([ec[m],
                                 np.arange(WIN - nfake, WIN, dtype=np.int64)])
            ld = np.concatenate([ed[m], np.zeros(nfake, dtype=np.int64)])
            nl = len(ls)
            assert nl <= lob * BLK, (w, nl)
            ilo[sb, g].reshape(-1)[:nl] = ls
            clo[sb, g].reshape(-1)[:nl] = lc.astype(np.float16)
            snlo[sb, g].reshape(-1)[:nl] = ls
            dnlo[sb, g].reshape(-1)[:nl] = ld
            hs, hc, hd = es[~m], ec[~m], ed[~m]
            nh = len(hs)
            assert nh <= hib * BLK, (w, nh)
            ihi[sb, g].reshape(-1)[:nh] = hs - SPLIT
            chi[sb, g].reshape(-1)[:nh] = hc.astype(np.float16)
            snhi[sb, g].reshape(-1)[:nh] = hs
            dnhi[sb, g].reshape(-1)[:nh] = hd

        # gather idx per pair: [NPAIR, 128, (2*nlo + 2*nhi)/16]
        sidx = np.empty((NPAIR, 128, (2 * cfg.nlo + 2 * cfg.nhi) // 16),
                        dtype=np.int16)
        for p in range(NPAIR):
            vlo = ilo[2 * p:2 * p + 2].reshape(-1)
            vhi = ihi[2 * p:2 * p + 2].reshape(-1)
            sidx[p, :, :2 * cfg.nlo // 16] = _wrap16(vlo)
            sidx[p, :, 2 * cfg.nlo // 16:] = _wrap16(vhi)

        # colidx [NSB, 128, NBLK] (block-col major: 4*lob lo | 4*hib hi)
        cx = np.empty((NSB, 128, cfg.nblk), dtype=np.float16)
        sn = np.empty((NSB, 128, cfg.nblk), dtype=np.int64)
        dn = np.empty((NSB, 128, cfg.nblk), dtype=np.int64)
        nl4 = 4 * lob
        for sb in range(NSB):
            cx[sb, :, :nl4] = clo[sb].reshape(nl4, BLK).T
            cx[sb, :, nl4:] = chi[sb].reshape(4 * hib, BLK).T
            sn[sb, :, :nl4] = snlo[sb].reshape(nl4, BLK).T
            sn[sb, :, nl4:] = snhi[sb].reshape(4 * hib, BLK).T
            dn[sb, :, :nl4] = dnlo[sb].reshape(nl4, BLK).T
            dn[sb, :, nl4:] = dnhi[sb].reshape(4 * hib, BLK).T
        per_core.append(dict(sidx=sidx, cx=cx, sn=sn, dn=dn))
    return cfg, per_core, new_row


# -------------------------------------------------------------- device side

def build_nc1(n_cores=N_CORES):
    """Launch 1: asd [104, 512] = per-node (a_src[4], a_dst[4]) for the
    core's 6250-node slab (padded to 6272), from xT fp16 and wad8."""
    nc = bacc.Bacc("TRN2", target_bir_lowering=False, debug=False,
                   num_devices=n_cores)
    xs_t = nc.dram_tensor("xT_slab", [128, 6272], F16, kind="ExternalInput")
    wad_t = nc.dram_tensor("wad8", [IN_DIM, 2 * HEADS], F16,
                           kind="ExternalInput")
    out_t = nc.dram_tensor("adstv", [2 * HEADS, 6656], F32,
                           kind="ExternalOutput")
    with tile.TileContext(nc) as tc:
        with (tc.tile_pool(name="c", bufs=1) as cp,
              tc.tile_pool(name="p", bufs=2, space="PSUM") as pp):
            wad_c = cp.tile([IN_DIM, 2 * HEADS], F16)
            nc.sync.dma_start(wad_c[:], wad_t.ap())
            xc = cp.tile([128, 6272], F16)
            nc.sync.dma_start(xc[:], xs_t.ap())
            acc = cp.tile([2 * HEADS, 6656], F32)
            nc.vector.memset(acc[:], 0.0)
            for k in range(13):
                n0 = 512 * k
                nk = min(512, 6272 - n0)
                ps = pp.tile([2 * HEADS, 512], F32, tag="ps")
                nc.tensor.matmul(ps[:, :nk], lhsT=wad_c[:],
                                 rhs=xc[:, n0:n0 + nk], start=True, stop=True)
                nc.scalar.copy(acc[:, n0:n0 + nk], ps[:, :nk])
            nc.sync.dma_start(out_t.ap(), acc[:])
    nc.compile()
    return nc


def build_nc2(cfg: Cfg):
    nc = bacc.Bacc("TRN2", target_bir_lowering=False, debug=False,
                   num_devices=cfg.n_cores)
    lob, hib, nblk = cfg.lob, cfg.hib, cfg.nblk
    nl4 = 4 * lob
    NLO2, NHI2 = 2 * cfg.nlo, 2 * cfg.nhi

    xtab_t = nc.dram_tensor("xtab", [NPAD, IN_DIM], F16, kind="ExternalInput")
    w_t = nc.dram_tensor("w_pd", [IN_DIM, HEADS * OUT_DIM], F16,
                         kind="ExternalInput")
    idp_t = nc.dram_tensor("ident_pd", [128, 128], F16, kind="ExternalInput")
    iota_t = nc.dram_tensor("iota_pd", [128, WIN, nblk], F16,
                            kind="ExternalInput")
    biasb_t = nc.dram_tensor("biasb", [128, HEADS * OUT_DIM], F32,
                             kind="ExternalInput")
    sidx_t = nc.dram_tensor("sidx", [NPAIR, 128, (NLO2 + NHI2) // 16], I16,
                            kind="ExternalInput")
    casd_t = nc.dram_tensor("casd", [NPAIR, 128, 2, 1 + 2 * HEADS, nblk],
                            F16, kind="ExternalInput")
    out_t = nc.dram_tensor("out", [WPC * WIN, HEADS * OUT_DIM], F16,
                           kind="ExternalOutput")

    with tile.TileContext(nc) as tc:
        with tc.tile_pool(name="consts", bufs=1) as cpool:
            w_c = cpool.tile([IN_DIM, HEADS * OUT_DIM], F16)
            nc.sync.dma_start(w_c[:], w_t.ap())
            idp_c = cpool.tile([128, 128], F16)
            nc.sync.dma_start(idp_c[:], idp_t.ap())
            iota_c = cpool.tile([128, WIN, nblk], F16)
            nc.sync.dma_start(iota_c[:].rearrange("p c b -> p (c b)"),
                              iota_t.ap().rearrange("p c b -> p (c b)"))
            biasb_c = cpool.tile([128, HEADS * OUT_DIM], F32)
            nc.sync.dma_start(biasb_c[:], biasb_t.ap())

            with (tc.tile_pool(name="st", bufs=3) as stp,
                  tc.tile_pool(name="g", bufs=3) as gp,
                  tc.tile_pool(name="al", bufs=3) as ap,
                  tc.tile_pool(name="m", bufs=3) as mp,
                  tc.tile_pool(name="z", bufs=3) as zp,
                  tc.tile_pool(name="ups", bufs=2, space="PSUM") as up,
                  tc.tile_pool(name="dps", bufs=2, space="PSUM") as dp,
                  tc.tile_pool(name="tps", bufs=2, space="PSUM") as tp,
                  tc.tile_pool(name="ops", bufs=2, space="PSUM") as op):
                for pair in range(NPAIR):
                    sidx = stp.tile([128, (NLO2 + NHI2) // 16], I16, tag="si")
                    nc.sync.dma_start(sidx[:], sidx_t.ap()[pair])
                    glo = gp.tile([128, NLO2 // BLK, IN_DIM], F16, tag="glo")
                    nc.gpsimd.dma_gather(glo[:], xtab_t.ap()[0:SPLIT, :],
                                         sidx[:, :NLO2 // 16], NLO2, NLO2,
                                         IN_DIM, single_packet=False)
                    ghi = gp.tile([128, NHI2 // BLK, IN_DIM], F16, tag="ghi")
                    nc.gpsimd.dma_gather(ghi[:], xtab_t.ap()[SPLIT:NPAD, :],
                                         sidx[:, NLO2 // 16:], NHI2, NHI2,
                                         IN_DIM, single_packet=False)
                    casd = stp.tile([128, 2, 1 + 2 * HEADS, nblk], F16,
                                    tag="casd")
                    nc.sync.dma_start(
                        casd[:].rearrange("p s e b -> p (s e b)"),
                        casd_t.ap()[pair].rearrange("p s e b -> p (s e b)"))
                    otp = zp.tile([128, 2, 128], F16, tag="otp")
                    for s in range(2):
                        sb = 2 * pair + s
                        cx = casd[:, s, 0, :]
                        asd = casd[:, s, 1:1 + 2 * HEADS, :]

                        asum = ap.tile([128, HEADS, nblk], F16, tag="asum")
                        nc.vector.tensor_tensor(out=asum[:],
                                                in0=asd[:, 0:HEADS, :],
                                                in1=asd[:, HEADS:2 * HEADS, :],
                                                op=mybir.AluOpType.add)
                        alr = ap.tile([128, HEADS, nblk], F16, tag="alr")
                        nc.scalar.activation(alr[:], asum[:],
                                             mybir.ActivationFunctionType.Prelu,
                                             alpha=LEAKY_SLOPE)
                        expa = ap.tile([128, HEADS, nblk], F16, tag="expa")
                        nc.scalar.activation(expa[:], alr[:],
                                             mybir.ActivationFunctionType.Exp)

                        oneh = mp.tile([128, WIN, nblk], F16, tag="oneh")
                        nc.vector.tensor_tensor(
                            out=oneh[:],
                            in0=iota_c[:],
                            in1=cx.unsqueeze(1)
                                .to_broadcast([128, WIN, nblk]),
                            op=mybir.AluOpType.is_equal)
                        mw = mp.tile([128, HEADS, WIN, nblk], F16, tag="mw")
                        nc.vector.tensor_tensor(
                            out=mw[:],
                            in0=oneh[:].unsqueeze(1)
                                .to_broadcast([128, HEADS, WIN, nblk]),
                            in1=expa[:].unsqueeze(2)
                                .to_broadcast([128, HEADS, WIN, nblk]),
                            op=mybir.AluOpType.mult)

                        zsb = zp.tile([128, 128], F32, tag="zsb")
                        dsb = zp.tile([128, HEADS], F32, tag="dsb")
                        for g in range(4):
                            U = up.tile([128, 128], F32, tag="U")
                            dn = dp.tile([WIN, HEADS], F32, tag="dn")
                            cols = ([g * lob + j for j in range(lob)]
                                    + [nl4 + g * hib + j for j in range(hib)])
                            for j, col in enumerate(cols):
                                if col < nl4:
                                    rows = glo[:, s * nl4 + col, :]
                                else:
                                    rows = ghi[:, s * 4 * hib + (col - nl4), :]
                                nc.tensor.matmul(U[:], lhsT=mw[:, :, :, col],
                                                 rhs=rows, start=(j == 0),
                                                 stop=(j == len(cols) - 1))
                            for j, col in enumerate(cols):
                                nc.tensor.matmul(dn[:], lhsT=oneh[:, :, col],
                                                 rhs=expa[:, :, col],
                                                 start=(j == 0),
                                                 stop=(j == len(cols) - 1))
                            usb = zp.tile([128, 128], F16, tag="usb")
                            nc.scalar.copy(usb[:], U[:])
                            ut_ps = tp.tile([128, 128], F16, tag="utp")
                            nc.tensor.transpose(out=ut_ps[:], in_=usb[:],
                                                identity=idp_c[:])
                            ut = zp.tile([128, 128], F16, tag="ut")
                            nc.vector.tensor_copy(ut[:], ut_ps[:])
                            o2 = op.tile([WIN, 128], F32, tag="o2")
                            for h in range(HEADS):
                                sl = slice(h * OUT_DIM, (h + 1) * OUT_DIM)
                                nc.tensor.matmul(o2[:, sl], lhsT=ut[:, sl],
                                                 rhs=w_c[:, sl],
                                                 start=True, stop=True)
                            nc.scalar.copy(zsb[g * WIN:(g + 1) * WIN, :],
                                           o2[:])
                            nc.scalar.copy(dsb[g * WIN:(g + 1) * WIN, :],
                                           dn[:])

                        rec = zp.tile([128, HEADS], F32, tag="rec")
                        nc.vector.reciprocal(rec[:], dsb[:])
                        zt = zp.tile([128, HEADS, OUT_DIM], F16, tag="zt")
                        nc.vector.tensor_tensor(
                            out=zt[:],
                            in0=zsb[:].rearrange("p (h d) -> p h d", h=HEADS),
                            in1=rec[:].unsqueeze(2)
                                .to_broadcast([128, HEADS, OUT_DIM]),
                            op=mybir.AluOpType.mult)
                        ztf = zt[:].rearrange("p h d -> p (h d)")
                        if cfg.bias_nonzero:
                            zt2 = zp.tile([128, 128], F16, tag="zt2")
                            nc.vector.tensor_tensor(out=zt2[:], in0=ztf,
                                                    in1=biasb_c[:],
                                                    op=mybir.AluOpType.add)
                            ztf = zt2[:]
                        # sigmoid via exp (stays in the exp act-table set):
                        # sg = 1/(1+exp(-z))
                        en = zp.tile([128, 128], F16, tag="en")
                        nc.scalar.activation(
                            en[:], ztf, mybir.ActivationFunctionType.Exp,
                            scale=-1.0)
                        den = zp.tile([128, 128], F32, tag="den")
                        nc.vector.tensor_scalar(den[:], en[:], 1.0, None,
                                                mybir.AluOpType.add)
                        sg = zp.tile([128, 128], F32, tag="sg")
                        nc.vector.reciprocal(sg[:], den[:])
                        mix = zp.tile([128, 128], F16, tag="mix")
                        nc.vector.tensor_scalar(mix[:], sg[:], CMIX - BETA,
                                                BETA, mybir.AluOpType.mult,
                                                mybir.AluOpType.add)
                        nc.vector.tensor_tensor(out=otp[:, s, :], in0=ztf,
                                                in1=mix[:],
                                                op=mybir.AluOpType.mult)
                    nc.sync.dma_start(
                        out_t.ap()[256 * pair:256 * pair + 256, :]
                            .rearrange("(r s) f -> r (s f)", s=2),
                        otp[:].rearrange("p s f -> p (s f)"))
    nc.compile()
    return nc


# ---------------------------------------------------------------- the API

def run(x, edge_index, W, att_src, att_dst, bias, n_cores=N_CORES,
        trace=False, trace_dir=None):
    x = np.asarray(x, dtype=np.float32)
    W32 = np.asarray(W, dtype=np.float32)
    att_src = np.asarray(att_src, dtype=np.float32)
    att_dst = np.asarray(att_dst, dtype=np.float32)
    bias = np.asarray(bias, dtype=np.float32)
    H, D = att_src.shape

    cfg, per_core, new_row = preprocess(edge_index, n_cores)
    cfg.bias_nonzero = bool(np.any(bias))

    # host-side param-only math + layout casts
    as4 = np.zeros((H * D, 2 * H), dtype=np.float32)
    for h in range(H):
        as4[h * D:(h + 1) * D, h] = att_src[h]
        as4[h * D:(h + 1) * D, H + h] = att_dst[h]
    wad8 = (W32 @ as4).astype(np.float16)
    xtab = np.zeros((NPAD, IN_DIM), dtype=np.float16)
    xtab[:N_NODES] = x.astype(np.float16)
    xT = np.ascontiguousarray(xtab.T)                  # [128, NPAD] fp16
    ident = np.eye(128, dtype=np.float16)
    # iota2[p, c, b] = c  (pre-expanded so the one-hot TT has packed operands)
    iota = np.broadcast_to(
        np.arange(WIN, dtype=np.float16)[None, :, None],
        (128, WIN, cfg.nblk)).copy()
    biasb = np.tile(bias, (128, 1)).astype(np.float32)

    tkw = {}
    tmp1 = tmp2 = None
    if trace:
        tkw = dict(trace=True, trace_cores=list(range(n_cores)))
        if trace_dir:
            tmp1 = os.path.join(trace_dir, "l1")
            tmp2 = os.path.join(trace_dir, "l2")
            os.makedirs(tmp1, exist_ok=True)
            os.makedirs(tmp2, exist_ok=True)

    # launch 1: per-node a_src/a_dst
    npc0 = N_NODES // n_cores
    nc1 = build_nc1(n_cores)
    in_maps1 = []
    for c in range(n_cores):
        slab = np.zeros((128, 6272), dtype=np.float16)
        slab[:, :npc0] = xT[:, c * npc0:(c + 1) * npc0]
        in_maps1.append(dict(xT_slab=slab, wad8=wad8))
    res1 = run_bass_kernel_spmd(nc1, in_maps1, core_ids=list(range(n_cores)),
                                tmpdir=tmp1, **tkw)
    asd8 = np.concatenate(
        [res1.results[c]["adstv"].T[:npc0] for c in range(n_cores)], axis=0)

    # host expansion of per-edge a_src/a_dst (indexing only)
    nc2 = build_nc2(cfg)
    in_maps = []
    for c in range(n_cores):
        pc = per_core[c]
        asd_pe = np.concatenate(
            [asd8[pc["sn"], 0:H], asd8[pc["dn"], H:2 * H]],
            axis=-1).transpose(0, 1, 3, 2)         # [NSB, 128, 8, nblk]
        casd = np.empty((NPAIR, 128, 2, 1 + 2 * H, cfg.nblk),
                        dtype=np.float16)
        casd[:, :, :, 0, :] = pc["cx"].reshape(NPAIR, 2, 128,
                                               cfg.nblk).transpose(0, 2, 1, 3)
        casd[:, :, :, 1:, :] = asd_pe.reshape(
            NPAIR, 2, 128, 2 * H, cfg.nblk).transpose(0, 2, 1, 3, 4)
        in_maps.append(dict(xtab=xtab, w_pd=W32.astype(np.float16),
                            ident_pd=ident, iota_pd=iota, biasb=biasb,
                            sidx=pc["sidx"], casd=casd))
    res = run_bass_kernel_spmd(nc2, in_maps, core_ids=list(range(n_cores)),
                               tmpdir=tmp2, **tkw)
    allout = np.concatenate([res.results[c]["out"] for c in range(n_cores)],
                            axis=0)                    # [51200, 128] fp16
    out = allout[new_row].astype(np.float32)
    parts = dict(nc1=nc1, in_maps1=in_maps1, nc2=nc2, in_maps2=in_maps,
                 res1=res1, res2=res, n_cores=n_cores, cfg=cfg)
    return out, parts


def kernel(**inputs) -> np.ndarray:
    out, _ = run(inputs["x"], inputs["edge_index"], inputs["W"],
                 inputs["att_src"], inputs["att_dst"], inputs["bias"])
    return out


# revision 29
# speedup vs baseline: 1.0535x; 1.0535x over previous
# MixGAT layer (GATConv + beta-mix swish) on 8 Trainium2 NeuronCores.
#
# Strategy (dst-node sharding): nodes are packed into fixed 32-dst "windows"
# (2D bin-packing by lo/hi in-degree so every window fits a static block
# budget); 200 windows per core.  Per superblock (4 windows = 128 dsts):
#   - dma_gather pulls each edge's RAW x row (fp16, 256B) from a host-cast
#     table; aggregation of raw x is exchanged with the W projection
#     (out = (sum_e w_e x[src]) @ W per head).
#   - alpha = lrelu(a_src[src]+a_dst[dst]) -> exp on ACT; a_src/a_dst are
#     per-node values computed on device in launch 1 and expanded per-edge
#     on host (indexing only).
#   - mw[e,(h,c)] = onehot(dst slot) * expa[e,h]; one matmul per 128-edge
#     block accumulates U[(h,c), f] = sum_e mw * x[src]; a second tiny
#     matmul accumulates the softmax denominators dn[c,h].
#   - U is transposed on the PE and projected through W per head, giving
#     node-major z rows; swish postproc and one contiguous output DMA per
#     128 dsts.
#
# kernel(**inputs) is self-contained: preprocessing is pure numpy indexing,
# the device kernels run via run_bass_kernel_spmd on cores 0-7.

import os

import numpy as np

import concourse.bass as bass
import concourse.mybir as mybir
import concourse.tile as tile
from concourse import bacc
from concourse.bass_utils import run_bass_kernel_spmd

F32 = mybir.dt.float32
F16 = mybir.dt.float16
I16 = mybir.dt.int16

# problem constants
N_NODES = 50000
IN_DIM = 128
HEADS = 4
OUT_DIM = 32
LEAKY_SLOPE = 0.2
BETA = 0.5
CMIX = 1.2
N_CORES = 8

# static schedule constants
WIN = 32          # dsts per window / group
BLK = 128         # edges per block (matmul contraction)
WPC = 200         # windows per core
NSB = WPC // 4    # superblocks (128 dsts) per core
NPAIR = NSB // 2  # gather pairs (2 superblocks per gather)
SPLIT = 32768     # int16-addressable table split
NPAD = 50176      # padded table rows (multiple of 128)
DEAD = 100.0      # colidx value for dead slots


class Cfg:
    def __init__(self, lob, hib, bias_nonzero=False, n_cores=N_CORES):
        self.lob = lob                  # lo blocks per window
        self.hib = hib                  # hi blocks per window
        self.nblk = 4 * (lob + hib)     # block columns per superblock
        self.nlo = 4 * lob * BLK        # lo slots per superblock
        self.nhi = 4 * hib * BLK
        self.bias_nonzero = bias_nonzero
        self.n_cores = n_cores


# ---------------------------------------------------------------- host side

def _wrap16(v):
    """idx vector [S*16] -> dma_gather idx layout [128, S] int16."""
    s = v.reshape(-1, 16).T                      # [16, S]
    return np.tile(s, (8, 1)).astype(np.int16)   # [128, S]


def assign_windows(deg_lo, deg_hi, lo_cap, hi_cap, n_win):
    """Snake-deal nodes to windows by hi-degree, then repair cap violations.
    Returns win_of[n]. Raises if infeasible under the caps."""
    n = len(deg_lo)
    order = np.argsort(-deg_hi, kind="stable")
    idx = np.arange(n)
    row = idx // n_win
    k = idx % n_win
    w = np.where(row % 2 == 0, k, n_win - 1 - k)
    win_of = np.empty(n, dtype=np.int64)
    win_of[order] = w

    cnt = np.bincount(win_of, minlength=n_win)
    assert cnt.max() <= WIN
    # fakes for empty slots count against lo capacity
    losum = np.bincount(win_of, weights=deg_lo, minlength=n_win) + (WIN - cnt)
    hisum = np.bincount(win_of, weights=deg_hi, minlength=n_win)

    # windows -> node lists for swapping
    by_win = [[] for _ in range(n_win)]
    for node in range(n):
        by_win[win_of[node]].append(node)

    def try_fix(sums, caps, other, ocap, deg, odeg):
        for _ in range(20000):
            wbad = int(np.argmax(sums))
            if sums[wbad] <= caps:
                return True
            # heaviest node in the bad window
            nodes_bad = by_win[wbad]
            a = max(nodes_bad, key=lambda x: deg[x])
            done = False
            for wgood in np.argsort(sums)[:64]:
                wgood = int(wgood)
                if wgood == wbad:
                    continue
                for b in sorted(by_win[wgood], key=lambda x: deg[x])[:8]:
                    d_s = deg[a] - deg[b]
                    d_o = odeg[a] - odeg[b]
                    if (sums[wbad] - d_s <= caps and sums[wgood] + d_s <= caps
                            and other[wbad] - d_o <= ocap
                            and other[wgood] + d_o <= ocap):
                        by_win[wbad].remove(a)
                        by_win[wgood].remove(b)
                        by_win[wbad].append(b)
                        by_win[wgood].append(a)
                        win_of[a], win_of[b] = wgood, wbad
                        sums[wbad] -= d_s
                        sums[wgood] += d_s
                        other[wbad] -= d_o
                        other[wgood] += d_o
                        done = True
                        break
                if done:
                    break
            if not done:
                return False
        return False

    if not try_fix(hisum, hi_cap, losum, lo_cap, deg_hi, deg_lo):
        raise RuntimeError("hi repair failed")
    if not try_fix(losum, lo_cap, hisum, hi_cap, deg_lo, deg_hi):
        raise RuntimeError("lo repair failed")
    assert losum.max() <= lo_cap and hisum.max() <= hi_cap
    return win_of


def preprocess(edge_index, n_cores=N_CORES):
    """Window assignment + static per-core gather/colidx/edge-stream arrays.

    Returns (cfg, per_core list, new_row[n] output permutation,
    sn/dn index arrays for the asd expansion)."""
    src0 = np.asarray(edge_index[0], dtype=np.int64)
    dst0 = np.asarray(edge_index[1], dtype=np.int64)
    loop = np.arange(N_NODES, dtype=np.int64)
    src = np.concatenate([src0, loop])
    dst = np.concatenate([dst0, loop])
    lo_mask_e = src < SPLIT
    deg_lo = np.bincount(dst[lo_mask_e], minlength=N_NODES)
    deg_hi = np.bincount(dst[~lo_mask_e], minlength=N_NODES)

    n_win = n_cores * WPC
    cfg = None
    win_of = None
    for lob, hib in ((6, 3), (7, 3), (7, 4), (8, 4)):
        try:
            win_of = assign_windows(deg_lo, deg_hi, lob * BLK, hib * BLK, n_win)
            cfg = Cfg(lob, hib)
            break
        except RuntimeError:
            continue
    assert cfg is not None, "window packing failed at all cap levels"
    lob, hib = cfg.lob, cfg.hib

    # slot assignment within windows (order of appearance)
    order = np.argsort(win_of, kind="stable")
    slot = np.empty(N_NODES, dtype=np.int64)
    bounds = np.searchsorted(win_of[order], np.arange(n_win + 1))
    for w in range(n_win):
        seg = order[bounds[w]:bounds[w + 1]]
        slot[seg] = np.arange(len(seg))
    # output row of each original node; the per-pair output DMA interleaves
    # the two superblocks of a pair: row = 256*pair + 2*(g*32+slot) + s
    core_of = win_of // WPC
    wl = win_of % WPC
    sb_of, g_of = wl // 4, wl % 4
    new_row = (core_of * (WPC * WIN) + (sb_of // 2) * 256
               + 2 * (g_of * WIN + slot) + (sb_of % 2))

    # per-edge: window / slot of the dst
    e_win = win_of[dst]
    e_slot = slot[dst]
    e_order = np.argsort(e_win, kind="stable")
    eb = np.searchsorted(e_win[e_order], np.arange(n_win + 1))

    per_core = []
    for c in range(n_cores):
        ilo = np.zeros((NSB, 4, lob, BLK), dtype=np.int64)
        ihi = np.zeros((NSB, 4, hib, BLK), dtype=np.int64)
        clo = np.full((NSB, 4, lob, BLK), DEAD, dtype=np.float16)
        chi = np.full((NSB, 4, hib, BLK), DEAD, dtype=np.float16)
        snlo = np.zeros((NSB, 4, lob, BLK), dtype=np.int64)
        dnlo = np.zeros((NSB, 4, lob, BLK), dtype=np.int64)
        snhi = np.zeros((NSB, 4, hib, BLK), dtype=np.int64)
        dnhi = np.zeros((NSB, 4, hib, BLK), dtype=np.int64)
        for wl in range(WPC):
            w = c * WPC + wl
            sb, g = wl // 4, wl % 4
            seg = e_order[eb[w]:eb[w + 1]]
            es, ec, ed = src[seg], e_slot[seg], dst[seg]
            m = es < SPLIT
            # lo side, with fake self-edges for empty slots
            nfake = WIN - (bounds[w + 1] - bounds[w])
            ls = np.concatenate([es[m], np.zeros(nfake, dtype=np.int64)])
            lc = np.concatenate([ec[m],
                                 np.arange(WIN - nfake, WIN, dtype=np.int64)])
            ld = np.concatenate([ed[m], np.zeros(nfake, dtype=np.int64)])
            nl = len(ls)
            assert nl <= lob * BLK, (w, nl)
            ilo[sb, g].reshape(-1)[:nl] = ls
            clo[sb, g].reshape(-1)[:nl] = lc.astype(np.float16)
            snlo[sb, g].reshape(-1)[:nl] = ls
            dnlo[sb, g].reshape(-1)[:nl] = ld
            hs, hc, hd = es[~m], ec[~m], ed[~m]
            nh = len(hs)
            assert nh <= hib * BLK, (w, nh)
            ihi[sb, g].reshape(-1)[:nh] = hs - SPLIT
            chi[sb, g].reshape(-1)[:nh] = hc.astype(np.float16)
            snhi[sb, g].reshape(-1)[:nh] = hs
            dnhi[sb, g].reshape(-1)[:nh] = hd

        # gather idx per pair: [NPAIR, 128, (2*nlo + 2*nhi)/16]
        sidx = np.empty((NPAIR, 128, (2 * cfg.nlo + 2 * cfg.nhi) // 16),
                        dtype=np.int16)
        for p in range(NPAIR):
            vlo = ilo[2 * p:2 * p + 2].reshape(-1)
            vhi = ihi[2 * p:2 * p + 2].reshape(-1)
            sidx[p, :, :2 * cfg.nlo // 16] = _wrap16(vlo)
            sidx[p, :, 2 * cfg.nlo // 16:] = _wrap16(vhi)

        # colidx [NSB, 128, NBLK] (block-col major: 4*lob lo | 4*hib hi)
        cx = np.empty((NSB, 128, cfg.nblk), dtype=np.float16)
        sn = np.empty((NSB, 128, cfg.nblk), dtype=np.int64)
        dn = np.empty((NSB, 128, cfg.nblk), dtype=np.int64)
        nl4 = 4 * lob
        for sb in range(NSB):
            cx[sb, :, :nl4] = clo[sb].reshape(nl4, BLK).T
            cx[sb, :, nl4:] = chi[sb].reshape(4 * hib, BLK).T
            sn[sb, :, :nl4] = snlo[sb].reshape(nl4, BLK).T
            sn[sb, :, nl4:] = snhi[sb].reshape(4 * hib, BLK).T
            dn[sb, :, :nl4] = dnlo[sb].reshape(nl4, BLK).T
            dn[sb, :, nl4:] = dnhi[sb].reshape(4 * hib, BLK).T
        per_core.append(dict(sidx=sidx, cx=cx, sn=sn, dn=dn))
    return cfg, per_core, new_row


# -------------------------------------------------------------- device side

def build_nc1(n_cores=N_CORES):
    """Launch 1: asd [104, 512] = per-node (a_src[4], a_dst[4]) for the
    core's 6250-node slab (padded to 6272), from xT fp16 and wad8."""
    nc = bacc.Bacc("TRN2", target_bir_lowering=False, debug=False,
                   num_devices=n_cores)
    xs_t = nc.dram_tensor("xT_slab", [128, 6272], F16, kind="ExternalInput")
    wad_t = nc.dram_tensor("wad8", [IN_DIM, 2 * HEADS], F16,
                           kind="ExternalInput")
    out_t = nc.dram_tensor("adstv", [2 * HEADS, 6272], F32,
                           kind="ExternalOutput")
    with tile.TileContext(nc) as tc:
        with (tc.tile_pool(name="c", bufs=1) as cp,
              tc.tile_pool(name="p", bufs=2, space="PSUM") as pp):
            wad_c = cp.tile([IN_DIM, 2 * HEADS], F16)
            nc.sync.dma_start(wad_c[:], wad_t.ap())
            xc = cp.tile([128, 6272], F16)
            for k in range(13):
                n0 = 512 * k
                nk = min(512, 6272 - n0)
                nc.sync.dma_start(xc[:, n0:n0 + nk], xs_t.ap()[:, n0:n0 + nk])
            acc = cp.tile([2 * HEADS, 6272], F32)
            for k in range(13):
                n0 = 512 * k
                nk = min(512, 6272 - n0)
                ps = pp.tile([2 * HEADS, 512], F32, tag="ps")
                nc.tensor.matmul(ps[:, :nk], lhsT=wad_c[:],
                                 rhs=xc[:, n0:n0 + nk], start=True, stop=True)
                if k % 2 == 0:
                    nc.scalar.copy(acc[:, n0:n0 + nk], ps[:, :nk])
                else:
                    nc.vector.tensor_copy(acc[:, n0:n0 + nk], ps[:, :nk])
            nc.sync.dma_start(out_t.ap(), acc[:])
    nc.compile()
    return nc


def build_nc2(cfg: Cfg):
    nc = bacc.Bacc("TRN2", target_bir_lowering=False, debug=False,
                   num_devices=cfg.n_cores)
    lob, hib, nblk = cfg.lob, cfg.hib, cfg.nblk
    nl4 = 4 * lob
    NLO2, NHI2 = 2 * cfg.nlo, 2 * cfg.nhi

    xtab_t = nc.dram_tensor("xtab", [NPAD, IN_DIM], F16, kind="ExternalInput")
    w_t = nc.dram_tensor("w_pd", [IN_DIM, HEADS * OUT_DIM], F16,
                         kind="ExternalInput")
    idp_t = nc.dram_tensor("ident_pd", [128, 128], F16, kind="ExternalInput")
    iota_t = nc.dram_tensor("iota_pd", [128, WIN, nblk], F16,
                            kind="ExternalInput")
    biasb_t = nc.dram_tensor("biasb", [128, HEADS * OUT_DIM], F32,
                             kind="ExternalInput")
    sidx_t = nc.dram_tensor("sidx", [NPAIR, 128, (NLO2 + NHI2) // 16], I16,
                            kind="ExternalInput")
    casd_t = nc.dram_tensor("casd", [NPAIR, 128, 2, 1 + 2 * HEADS, nblk],
                            F16, kind="ExternalInput")
    out_t = nc.dram_tensor("out", [WPC * WIN, HEADS * OUT_DIM], F16,
                           kind="ExternalOutput")

    with tile.TileContext(nc) as tc:
        with tc.tile_pool(name="consts", bufs=1) as cpool:
            w_c = cpool.tile([IN_DIM, HEADS * OUT_DIM], F16)
            nc.sync.dma_start(w_c[:], w_t.ap())
            idp_c = cpool.tile([128, 128], F16)
            nc.sync.dma_start(idp_c[:], idp_t.ap())
            iota_c = cpool.tile([128, WIN, nblk], F16)
            nc.sync.dma_start(iota_c[:].rearrange("p c b -> p (c b)"),
                              iota_t.ap().rearrange("p c b -> p (c b)"))
            biasb_c = cpool.tile([128, HEADS * OUT_DIM], F32)
            nc.sync.dma_start(biasb_c[:], biasb_t.ap())

            with (tc.tile_pool(name="st", bufs=4) as stp,
                  tc.tile_pool(name="g", bufs=4) as gp,
                  tc.tile_pool(name="al", bufs=3) as ap,
                  tc.tile_pool(name="m", bufs=4) as mp,
                  tc.tile_pool(name="z", bufs=3) as zp,
                  tc.tile_pool(name="ups", bufs=2, space="PSUM") as up,
                  tc.tile_pool(name="dps", bufs=2, space="PSUM") as dp,
                  tc.tile_pool(name="tps", bufs=2, space="PSUM") as tp,
                  tc.tile_pool(name="ops", bufs=2, space="PSUM") as op):
                for pair in range(NPAIR):
                    sidx = stp.tile([128, (NLO2 + NHI2) // 16], I16, tag="si")
                    nc.sync.dma_start(sidx[:], sidx_t.ap()[pair])
                    glo = gp.tile([128, NLO2 // BLK, IN_DIM], F16, tag="glo")
                    nc.gpsimd.dma_gather(glo[:], xtab_t.ap()[0:SPLIT, :],
                                         sidx[:, :NLO2 // 16], NLO2, NLO2,
                                         IN_DIM, single_packet=False)
                    ghi = gp.tile([128, NHI2 // BLK, IN_DIM], F16, tag="ghi")
                    nc.gpsimd.dma_gather(ghi[:], xtab_t.ap()[SPLIT:NPAD, :],
                                         sidx[:, NLO2 // 16:], NHI2, NHI2,
                                         IN_DIM, single_packet=False)
                    casd = stp.tile([128, 2, 1 + 2 * HEADS, nblk], F16,
                                    tag="casd")
                    nc.sync.dma_start(
                        casd[:].rearrange("p s e b -> p (s e b)"),
                        casd_t.ap()[pair].rearrange("p s e b -> p (s e b)"))
                    otp = zp.tile([128, 2, 128], F16, tag="otp")
                    for s in range(2):
                        sb = 2 * pair + s
                        cx = casd[:, s, 0, :]
                        asd = casd[:, s, 1:1 + 2 * HEADS, :]

                        asum = ap.tile([128, HEADS, nblk], F16, tag="asum")
                        nc.vector.tensor_tensor(out=asum[:],
                                                in0=asd[:, 0:HEADS, :],
                                                in1=asd[:, HEADS:2 * HEADS, :],
                                                op=mybir.AluOpType.add)
                        alr = ap.tile([128, HEADS, nblk], F16, tag="alr")
                        nc.scalar.activation(alr[:], asum[:],
                                             mybir.ActivationFunctionType.Prelu,
                                             alpha=LEAKY_SLOPE)
                        expa = ap.tile([128, HEADS, nblk], F16, tag="expa")
                        nc.scalar.activation(expa[:], alr[:],
                                             mybir.ActivationFunctionType.Exp)

                        oneh = mp.tile([128, WIN, nblk], F16, tag="oneh")
                        nc.vector.tensor_tensor(
                            out=oneh[:],
                            in0=iota_c[:],
                            in1=cx.unsqueeze(1)
                                .to_broadcast([128, WIN, nblk]),
                            op=mybir.AluOpType.is_equal)
                        mw = mp.tile([128, HEADS, WIN, nblk], F16, tag="mw")
                        nc.vector.tensor_tensor(
                            out=mw[:],
                            in0=oneh[:].unsqueeze(1)
                                .to_broadcast([128, HEADS, WIN, nblk]),
                            in1=expa[:].unsqueeze(2)
                                .to_broadcast([128, HEADS, WIN, nblk]),
                            op=mybir.AluOpType.mult)

                        zsb = zp.tile([128, 128], F32, tag="zsb")
                        dsb = zp.tile([128, HEADS], F32, tag="dsb")
                        for g in range(4):
                            U = up.tile([128, 128], F32, tag="U")
                            dn = dp.tile([WIN, HEADS], F32, tag="dn")
                            cols = ([g * lob + j for j in range(lob)]
                                    + [nl4 + g * hib + j for j in range(hib)])
                            for j, col in enumerate(cols):
                                if col < nl4:
                                    rows = glo[:, s * nl4 + col, :]
                                else:
                                    rows = ghi[:, s * 4 * hib + (col - nl4), :]
                                nc.tensor.matmul(U[:], lhsT=mw[:, :, :, col],
                                                 rhs=rows, start=(j == 0),
                                                 stop=(j == len(cols) - 1))
                            for j, col in enumerate(cols):
                                nc.tensor.matmul(dn[:], lhsT=oneh[:, :, col],
                                                 rhs=expa[:, :, col],
                                                 start=(j == 0),
                                                 stop=(j == len(cols) - 1))
                            usb = zp.tile([128, 128], F16, tag="usb")
                            nc.scalar.copy(usb[:], U[:])
                            ut_ps = tp.tile([128, 128], F16, tag="utp")
                            nc.tensor.transpose(out=ut_ps[:], in_=usb[:],
                                                identity=idp_c[:])
                            ut = zp.tile([128, 128], F16, tag="ut")
                            nc.vector.tensor_copy(ut[:], ut_ps[:])
                            o2 = op.tile([WIN, 128], F32, tag="o2")
                            for h in range(HEADS):
                                sl = slice(h * OUT_DIM, (h + 1) * OUT_DIM)
                                nc.tensor.matmul(o2[:, sl], lhsT=ut[:, sl],
                                                 rhs=w_c[:, sl],
                                                 start=True, stop=True)
                            nc.scalar.copy(zsb[g * WIN:(g + 1) * WIN, :],
                                           o2[:])
                            nc.scalar.copy(dsb[g * WIN:(g + 1) * WIN, :],
                                           dn[:])

                        rec = zp.tile([128, HEADS], F32, tag="rec")
                        nc.vector.reciprocal(rec[:], dsb[:])
                        zt = zp.tile([128, HEADS, OUT_DIM], F16, tag="zt")
                        nc.vector.tensor_tensor(
                            out=zt[:],
                            in0=zsb[:].rearrange("p (h d) -> p h d", h=HEADS),
                            in1=rec[:].unsqueeze(2)
                                .to_broadcast([128, HEADS, OUT_DIM]),
                            op=mybir.AluOpType.mult)
                        ztf = zt[:].rearrange("p h d -> p (h d)")
                        if cfg.bias_nonzero:
                            zt2 = zp.tile([128, 128], F16, tag="zt2")
                            nc.vector.tensor_tensor(out=zt2[:], in0=ztf,
                                                    in1=biasb_c[:],
                                                    op=mybir.AluOpType.add)
                            ztf = zt2[:]
                        # sigmoid via exp (stays in the exp act-table set):
                        # sg = 1/(1+exp(-z))
                        en = zp.tile([128, 128], F16, tag="en")
                        nc.scalar.activation(
                            en[:], ztf, mybir.ActivationFunctionType.Exp,
                            scale=-1.0)
                        den = zp.tile([128, 128], F32, tag="den")
                        nc.vector.tensor_scalar(den[:], en[:], 1.0, None,
                                                mybir.AluOpType.add)
                        sg = zp.tile([128, 128], F32, tag="sg")
                        nc.vector.reciprocal(sg[:], den[:])
                        mix = zp.tile([128, 128], F16, tag="mix")
                        nc.vector.tensor_scalar(mix[:], sg[:], CMIX - BETA,
                                                BETA, mybir.AluOpType.mult,
                                                mybir.AluOpType.add)
                        nc.vector.tensor_tensor(out=otp[:, s, :], in0=ztf,
                                                in1=mix[:],
                                                op=mybir.AluOpType.mult)
                    nc.sync.dma_start(
                        out_t.ap()[256 * pair:256 * pair + 256, :]
                            .rearrange("(r s) f -> r (s f)", s=2),
                        otp[:].rearrange("p s f -> p (s f)"))
    nc.compile()
    return nc


# ---------------------------------------------------------------- the API

def run(x, edge_index, W, att_src, att_dst, bias, n_cores=N_CORES,
        trace=False, trace_dir=None):
    x = np.asarray(x, dtype=np.float32)
    W32 = np.asarray(W, dtype=np.float32)
    att_src = np.asarray(att_src, dtype=np.float32)
    att_dst = np.asarray(att_dst, dtype=np.float32)
    bias = np.asarray(bias, dtype=np.float32)
    H, D = att_src.shape

    cfg, per_core, new_row = preprocess(edge_index, n_cores)
    cfg.bias_nonzero = bool(np.any(bias))

    # host-side param-only math + layout casts
    as4 = np.zeros((H * D, 2 * H), dtype=np.float32)
    for h in range(H):
        as4[h * D:(h + 1) * D, h] = att_src[h]
        as4[h * D:(h + 1) * D, H + h] = att_dst[h]
    wad8 = (W32 @ as4).astype(np.float16)
    xtab = np.zeros((NPAD, IN_DIM), dtype=np.float16)
    xtab[:N_NODES] = x.astype(np.float16)
    xT = np.ascontiguousarray(xtab.T)                  # [128, NPAD] fp16
    ident = np.eye(128, dtype=np.float16)
    # iota2[p, c, b] = c  (pre-expanded so the one-hot TT has packed operands)
    iota = np.broadcast_to(
        np.arange(WIN, dtype=np.float16)[None, :, None],
        (128, WIN, cfg.nblk)).copy()
    biasb = np.tile(bias, (128, 1)).astype(np.float32)

    tkw = {}
    tmp1 = tmp2 = None
    if trace:
        tkw = dict(trace=True, trace_cores=list(range(n_cores)))
        if trace_dir:
            tmp1 = os.path.join(trace_dir, "l1")
            tmp2 = os.path.join(trace_dir, "l2")
            os.makedirs(tmp1, exist_ok=True)
            os.makedirs(tmp2, exist_ok=True)

    # launch 1: per-node a_src/a_dst
    npc0 = N_NODES // n_cores
    nc1 = build_nc1(n_cores)
    in_maps1 = []
    for c in range(n_cores):
        slab = np.zeros((128, 6272), dtype=np.float16)
        slab[:, :npc0] = xT[:, c * npc0:(c + 1) * npc0]
        in_maps1.append(dict(xT_slab=slab, wad8=wad8))
    res1 = run_bass_kernel_spmd(nc1, in_maps1, core_ids=list(range(n_cores)),
                                tmpdir=tmp1, **tkw)
    asd8 = np.concatenate(
        [res1.results[c]["adstv"].T[:npc0] for c in range(n_cores)], axis=0)

    # host expansion of per-edge a_src/a_dst (indexing only)
    nc2 = build_nc2(cfg)
    in_maps = []
    for c in range(n_cores):
        pc = per_core[c]
        asd_pe = np.concatenate(
            [asd8[pc["sn"], 0:H], asd8[pc["dn"], H:2 * H]],
            axis=-1).transpose(0, 1, 3, 2)         # [NSB, 128, 8, nblk]
        casd = np.empty((NPAIR, 128, 2, 1 + 2 * H, cfg.nblk),
                        dtype=np.float16)
        casd[:, :, :, 0, :] = pc["cx"].reshape(NPAIR, 2, 128,
                                               cfg.nblk).transpose(0, 2, 1, 3)
        casd[:, :, :, 1:, :] = asd_pe.reshape(
            NPAIR, 2, 128, 2 * H, cfg.nblk).transpose(0, 2, 1, 3, 4)
        in_maps.append(dict(xtab=xtab, w_pd=W32.astype(np.float16),
                            ident_pd=ident, iota_pd=iota, biasb=biasb,
                            sidx=pc["sidx"], casd=casd))
    res = run_bass_kernel_spmd(nc2, in_maps, core_ids=list(range(n_cores)),
                               tmpdir=tmp2, **tkw)
    allout = np.concatenate([res.results[c]["out"] for c in range(n_cores)],
                            axis=0)                    # [51200, 128] fp16
    out = allout[new_row].astype(np.float32)
    parts = dict(nc1=nc1, in_maps1=in_maps1, nc2=nc2, in_maps2=in_maps,
                 res1=res1, res2=res, n_cores=n_cores, cfg=cfg)
    return out, parts


def kernel(**inputs) -> np.ndarray:
    out, _ = run(inputs["x"], inputs["edge_index"], inputs["W"],
                 inputs["att_src"], inputs["att_dst"], inputs["bias"])
    return out


# revision 46
# speedup vs baseline: 1.0784x; 1.0237x over previous
# MixGAT layer (GATConv + beta-mix swish) on 8 Trainium2 NeuronCores.
#
# Strategy (dst-node sharding): nodes are packed into fixed 32-dst "windows"
# (2D bin-packing by lo/hi in-degree so every window fits a static block
# budget); 200 windows per core.  Per superblock (4 windows = 128 dsts):
#   - dma_gather pulls each edge's RAW x row (fp16, 256B) from a host-cast
#     table; aggregation of raw x is exchanged with the W projection
#     (out = (sum_e w_e x[src]) @ W per head).
#   - alpha = lrelu(a_src[src]+a_dst[dst]) -> exp on ACT; a_src/a_dst are
#     per-node values computed on device in launch 1 and expanded per-edge
#     on host (indexing only).
#   - mw[e,(h,c)] = onehot(dst slot) * expa[e,h]; one matmul per 128-edge
#     block accumulates U[(h,c), f] = sum_e mw * x[src]; a second tiny
#     matmul accumulates the softmax denominators dn[c,h].
#   - U is transposed on the PE and projected through W per head, giving
#     node-major z rows; swish postproc and one contiguous output DMA per
#     128 dsts.
#
# kernel(**inputs) is self-contained: preprocessing is pure numpy indexing,
# the device kernels run via run_bass_kernel_spmd on cores 0-7.

import os

import numpy as np

import concourse.bass as bass
import concourse.mybir as mybir
import concourse.tile as tile
from concourse import bacc
from concourse.bass_utils import run_bass_kernel_spmd

F32 = mybir.dt.float32
F16 = mybir.dt.float16
I16 = mybir.dt.int16

# problem constants
N_NODES = 50000
IN_DIM = 128
HEADS = 4
OUT_DIM = 32
LEAKY_SLOPE = 0.2
BETA = 0.5
CMIX = 1.2
N_CORES = 8

# static schedule constants
WIN = 32          # dsts per window / group
BLK = 128         # edges per block (matmul contraction)
WPC = 200         # windows per core
NSB = WPC // 4    # superblocks (128 dsts) per core
NPAIR = NSB // 2  # gather pairs (2 superblocks per gather)
SPLIT = 32768     # int16-addressable table split
NPAD = 50176      # padded table rows (multiple of 128)
DEAD = 100.0      # colidx value for dead slots


class Cfg:
    def __init__(self, lob, hib, bias_nonzero=False, n_cores=N_CORES,
                 n_light=0):
        self.lob = lob                  # max lo blocks per window
        self.hib = hib                  # hi blocks per window
        self.nblk = 4 * (lob + hib)     # block columns per superblock (max)
        self.nlo = 4 * lob * BLK        # lo slots per superblock (max)
        self.nhi = 4 * hib * BLK
        self.bias_nonzero = bias_nonzero
        self.n_cores = n_cores
        self.n_light = n_light  # leading superblocks with one fewer lo block
        self.lob_sb = [lob - 1] * n_light + [lob] * (NSB - n_light)


# ---------------------------------------------------------------- host side

def _wrap16(v):
    """idx vector [S*16] -> dma_gather idx layout [128, S] int16."""
    s = v.reshape(-1, 16).T                      # [16, S]
    return np.tile(s, (8, 1)).astype(np.int16)   # [128, S]


def assign_windows(deg_lo, deg_hi, lo_cap, hi_cap, n_win, lo_caps=None):
    """Deal nodes to windows (hi balanced; lo shaped toward per-window
    targets when lo_caps is given), then repair cap violations.
    Returns win_of[n]. Raises if infeasible under the caps."""
    n = len(deg_lo)
    order = np.argsort(-deg_hi, kind="stable")
    win_of = np.empty(n, dtype=np.int64)
    if lo_caps is None:
        idx = np.arange(n)
        row = idx // n_win
        k = idx % n_win
        w = np.where(row % 2 == 0, k, n_win - 1 - k)
        win_of[order] = w
    else:
        # per-hi-rank rows: give the row's biggest lo-deg nodes to the
        # windows furthest below their (scaled) lo target
        lo_caps = np.asarray(lo_caps, dtype=np.float64)
        total = deg_lo.sum() + (n_win * WIN - n)     # + fakes
        target = lo_caps * (total / lo_caps.sum())
        cur = np.zeros(n_win)
        for k in range((n + n_win - 1) // n_win):
            nodes_k = order[k * n_win:(k + 1) * n_win]
            need_rank = np.argsort(cur - target, kind="stable")  # neediest 1st
            by_lo = nodes_k[np.argsort(-deg_lo[nodes_k], kind="stable")]
            wsel = need_rank[:len(by_lo)]
            win_of[by_lo] = wsel
            cur[wsel] += deg_lo[by_lo]

    cnt = np.bincount(win_of, minlength=n_win)
    assert cnt.max() <= WIN
    # fakes for empty slots count against lo capacity
    losum = np.bincount(win_of, weights=deg_lo, minlength=n_win) + (WIN - cnt)
    hisum = np.bincount(win_of, weights=deg_hi, minlength=n_win)
    locap = (np.full(n_win, lo_cap, dtype=np.float64) if lo_caps is None
             else np.asarray(lo_caps, dtype=np.float64))
    hicap = np.full(n_win, hi_cap, dtype=np.float64)

    # windows -> node lists for swapping
    by_win = [[] for _ in range(n_win)]
    for node in range(n):
        by_win[win_of[node]].append(node)

    def try_fix(sums, caps, other, ocap, deg, odeg):
        for _ in range(20000):
            over = sums - caps
            wbad = int(np.argmax(over))
            if over[wbad] <= 0:
                return True
            # heaviest node in the bad window
            a = max(by_win[wbad], key=lambda x: deg[x])
            done = False
            for wgood in np.argsort(over)[:64]:
                wgood = int(wgood)
                if wgood == wbad:
                    continue
                for b in sorted(by_win[wgood], key=lambda x: deg[x])[:8]:
                    d_s = deg[a] - deg[b]
                    d_o = odeg[a] - odeg[b]
                    if (sums[wbad] - d_s <= caps[wbad]
                            and sums[wgood] + d_s <= caps[wgood]
                            and other[wbad] - d_o <= ocap[wbad]
                            and other[wgood] + d_o <= ocap[wgood]):
                        by_win[wbad].remove(a)
                        by_win[wgood].remove(b)
                        by_win[wbad].append(b)
                        by_win[wgood].append(a)
                        win_of[a], win_of[b] = wgood, wbad
                        sums[wbad] -= d_s
                        sums[wgood] += d_s
                        other[wbad] -= d_o
                        other[wgood] += d_o
                        done = True
                        break
                if done:
                    break
            if not done:
                return False
        return False

    if not try_fix(hisum, hicap, losum, locap, deg_hi, deg_lo):
        raise RuntimeError("hi repair failed")
    if not try_fix(losum, locap, hisum, hicap, deg_lo, deg_hi):
        raise RuntimeError("lo repair failed")
    assert (losum <= locap).all() and (hisum <= hicap).all()
    return win_of


def preprocess(edge_index, n_cores=N_CORES):
    """Window assignment + static per-core gather/colidx/edge-stream arrays.

    Returns (cfg, per_core list, new_row[n] output permutation,
    sn/dn index arrays for the asd expansion)."""
    src0 = np.asarray(edge_index[0], dtype=np.int64)
    dst0 = np.asarray(edge_index[1], dtype=np.int64)
    loop = np.arange(N_NODES, dtype=np.int64)
    src = np.concatenate([src0, loop])
    dst = np.concatenate([dst0, loop])
    lo_mask_e = src < SPLIT
    deg_lo = np.bincount(dst[lo_mask_e], minlength=N_NODES)
    deg_hi = np.bincount(dst[~lo_mask_e], minlength=N_NODES)

    n_win = n_cores * WPC
    cfg = None
    win_of = None
    for lob, hib, n_light in ((6, 3, 32), (6, 3, 28), (6, 3, 24),
                              (6, 3, 0), (7, 3, 0), (7, 4, 0), (8, 4, 0)):
        lo_caps = None
        if n_light:
            # first n_light superblocks per core have lob-1 lo blocks
            wl = np.arange(n_win) % WPC
            lo_caps = np.where(wl // 4 < n_light, (lob - 1) * BLK,
                               lob * BLK).astype(np.float64)
        try:
            win_of = assign_windows(deg_lo, deg_hi, lob * BLK, hib * BLK,
                                    n_win, lo_caps=lo_caps)
            cfg = Cfg(lob, hib, n_light=n_light)
            break
        except RuntimeError:
            continue
    assert cfg is not None, "window packing failed at all cap levels"
    lob, hib = cfg.lob, cfg.hib

    # slot assignment within windows (order of appearance)
    order = np.argsort(win_of, kind="stable")
    slot = np.empty(N_NODES, dtype=np.int64)
    bounds = np.searchsorted(win_of[order], np.arange(n_win + 1))
    for w in range(n_win):
        seg = order[bounds[w]:bounds[w + 1]]
        slot[seg] = np.arange(len(seg))
    # output row of each original node; the per-pair output DMA interleaves
    # the two superblocks of a pair: row = 256*pair + 2*(g*32+slot) + s
    core_of = win_of // WPC
    wl = win_of % WPC
    sb_of, g_of = wl // 4, wl % 4
    new_row = (core_of * (WPC * WIN) + (sb_of // 2) * 256
               + 2 * (g_of * WIN + slot) + (sb_of % 2))

    # per-edge: window / slot of the dst
    e_win = win_of[dst]
    e_slot = slot[dst]
    e_order = np.argsort(e_win, kind="stable")
    eb = np.searchsorted(e_win[e_order], np.arange(n_win + 1))

    per_core = []
    for c in range(n_cores):
        ilo = np.zeros((NSB, 4, lob, BLK), dtype=np.int64)
        ihi = np.zeros((NSB, 4, hib, BLK), dtype=np.int64)
        clo = np.full((NSB, 4, lob, BLK), DEAD, dtype=np.float16)
        chi = np.full((NSB, 4, hib, BLK), DEAD, dtype=np.float16)
        snlo = np.zeros((NSB, 4, lob, BLK), dtype=np.int64)
        dnlo = np.zeros((NSB, 4, lob, BLK), dtype=np.int64)
        snhi = np.zeros((NSB, 4, hib, BLK), dtype=np.int64)
        dnhi = np.zeros((NSB, 4, hib, BLK), dtype=np.int64)
        for wl in range(WPC):
            w = c * WPC + wl
            sb, g = wl // 4, wl % 4
            lob_s = cfg.lob_sb[sb]
            seg = e_order[eb[w]:eb[w + 1]]
            es, ec, ed = src[seg], e_slot[seg], dst[seg]
            m = es < SPLIT
            # lo side, with fake self-edges for empty slots
            nfake = WIN - (bounds[w + 1] - bounds[w])
            ls = np.concatenate([es[m], np.zeros(nfake, dtype=np.int64)])
            lc = np.concatenate([ec[m],
                                 np.arange(WIN - nfake, WIN, dtype=np.int64)])
            ld = np.concatenate([ed[m], np.zeros(nfake, dtype=np.int64)])
            nl = len(ls)
            assert nl <= lob_s * BLK, (w, nl)
            ilo[sb, g].reshape(-1)[:nl] = ls
            clo[sb, g].reshape(-1)[:nl] = lc.astype(np.float16)
            snlo[sb, g].reshape(-1)[:nl] = ls
            dnlo[sb, g].reshape(-1)[:nl] = ld
            hs, hc, hd = es[~m], ec[~m], ed[~m]
            nh = len(hs)
            assert nh <= hib * BLK, (w, nh)
            ihi[sb, g].reshape(-1)[:nh] = hs - SPLIT
            chi[sb, g].reshape(-1)[:nh] = hc.astype(np.float16)
            snhi[sb, g].reshape(-1)[:nh] = hs
            dnhi[sb, g].reshape(-1)[:nh] = hd

        # gather idx per pair: [NPAIR, 128, (2*nlo + 2*nhi)/16] (zero pad)
        sidx = np.zeros((NPAIR, 128, (2 * cfg.nlo + 2 * cfg.nhi) // 16),
                        dtype=np.int16)
        for p in range(NPAIR):
            lob_p = cfg.lob_sb[2 * p]
            vlo = ilo[2 * p:2 * p + 2, :, :lob_p, :].reshape(-1)
            vhi = ihi[2 * p:2 * p + 2].reshape(-1)
            nc_lo = len(vlo) // 16
            sidx[p, :, :nc_lo] = _wrap16(vlo)
            sidx[p, :, nc_lo:nc_lo + len(vhi) // 16] = _wrap16(vhi)

        # colidx [NSB, 128, NBLK] (block-col major: 4*lob_s lo | 4*hib hi)
        cx = np.full((NSB, 128, cfg.nblk), DEAD, dtype=np.float16)
        sn = np.zeros((NSB, 128, cfg.nblk), dtype=np.int64)
        dn = np.zeros((NSB, 128, cfg.nblk), dtype=np.int64)
        for sb in range(NSB):
            lob_s = cfg.lob_sb[sb]
            nl4 = 4 * lob_s
            nh4 = 4 * hib
            cx[sb, :, :nl4] = clo[sb, :, :lob_s].reshape(nl4, BLK).T
            cx[sb, :, nl4:nl4 + nh4] = chi[sb].reshape(nh4, BLK).T
            sn[sb, :, :nl4] = snlo[sb, :, :lob_s].reshape(nl4, BLK).T
            sn[sb, :, nl4:nl4 + nh4] = snhi[sb].reshape(nh4, BLK).T
            dn[sb, :, :nl4] = dnlo[sb, :, :lob_s].reshape(nl4, BLK).T
            dn[sb, :, nl4:nl4 + nh4] = dnhi[sb].reshape(nh4, BLK).T
        per_core.append(dict(sidx=sidx, cx=cx, sn=sn, dn=dn))
    return cfg, per_core, new_row


# -------------------------------------------------------------- device side

def build_nc1(n_cores=N_CORES):
    """Launch 1: asd [104, 512] = per-node (a_src[4], a_dst[4]) for the
    core's 6250-node slab (padded to 6272), from xT fp16 and wad8."""
    nc = bacc.Bacc("TRN2", target_bir_lowering=False, debug=False,
                   num_devices=n_cores)
    xs_t = nc.dram_tensor("xT_slab", [128, 6272], F16, kind="ExternalInput")
    wad_t = nc.dram_tensor("wad8", [IN_DIM, 2 * HEADS], F16,
                           kind="ExternalInput")
    out_t = nc.dram_tensor("adstv", [2 * HEADS, 6272], F32,
                           kind="ExternalOutput")
    with tile.TileContext(nc) as tc:
        with (tc.tile_pool(name="c", bufs=1) as cp,
              tc.tile_pool(name="p", bufs=2, space="PSUM") as pp):
            wad_c = cp.tile([IN_DIM, 2 * HEADS], F16)
            nc.sync.dma_start(wad_c[:], wad_t.ap())
            xc = cp.tile([128, 6272], F16)
            for k in range(13):
                n0 = 512 * k
                nk = min(512, 6272 - n0)
                nc.sync.dma_start(xc[:, n0:n0 + nk], xs_t.ap()[:, n0:n0 + nk])
            acc = cp.tile([2 * HEADS, 6272], F32)
            for k in range(13):
                n0 = 512 * k
                nk = min(512, 6272 - n0)
                ps = pp.tile([2 * HEADS, 512], F32, tag="ps")
                nc.tensor.matmul(ps[:, :nk], lhsT=wad_c[:],
                                 rhs=xc[:, n0:n0 + nk], start=True, stop=True)
                if k % 2 == 0:
                    nc.scalar.copy(acc[:, n0:n0 + nk], ps[:, :nk])
                else:
                    nc.vector.tensor_copy(acc[:, n0:n0 + nk], ps[:, :nk])
            nc.sync.dma_start(out_t.ap(), acc[:])
    nc.compile()
    return nc


def build_nc2(cfg: Cfg):
    nc = bacc.Bacc("TRN2", target_bir_lowering=False, debug=False,
                   num_devices=cfg.n_cores)
    lob, hib, nblk = cfg.lob, cfg.hib, cfg.nblk
    nl4 = 4 * lob
    NLO2, NHI2 = 2 * cfg.nlo, 2 * cfg.nhi

    xtab_t = nc.dram_tensor("xtab", [NPAD, IN_DIM], F16, kind="ExternalInput")
    w_t = nc.dram_tensor("w_pd", [IN_DIM, HEADS * OUT_DIM], F16,
                         kind="ExternalInput")
    idp_t = nc.dram_tensor("ident_pd", [128, 128], F16, kind="ExternalInput")
    iota_t = nc.dram_tensor("iota_pd", [128, WIN, nblk], F16,
                            kind="ExternalInput")
    biasb_t = nc.dram_tensor("biasb", [128, HEADS * OUT_DIM], F32,
                             kind="ExternalInput")
    sidx_t = nc.dram_tensor("sidx", [NPAIR, 128, (NLO2 + NHI2) // 16], I16,
                            kind="ExternalInput")
    casd_t = nc.dram_tensor("casd", [NPAIR, 128, 2, 1 + 2 * HEADS, nblk],
                            F16, kind="ExternalInput")
    out_t = nc.dram_tensor("out", [WPC * WIN, HEADS * OUT_DIM], F16,
                           kind="ExternalOutput")

    with tile.TileContext(nc) as tc:
        with tc.tile_pool(name="consts", bufs=1) as cpool:
            w_c = cpool.tile([IN_DIM, HEADS * OUT_DIM], F16)
            nc.sync.dma_start(w_c[:], w_t.ap())
            idp_c = cpool.tile([128, 128], F16)
            nc.sync.dma_start(idp_c[:], idp_t.ap())
            iota_c = cpool.tile([128, WIN, nblk], F16)
            nc.sync.dma_start(iota_c[:].rearrange("p c b -> p (c b)"),
                                iota_t.ap().rearrange("p c b -> p (c b)"))
            biasb_c = cpool.tile([128, HEADS * OUT_DIM], F32)
            nc.sync.dma_start(biasb_c[:], biasb_t.ap())

            with (tc.tile_pool(name="st", bufs=4) as stp,
                  tc.tile_pool(name="g", bufs=4) as gp,
                  tc.tile_pool(name="al", bufs=3) as ap,
                  tc.tile_pool(name="m", bufs=4) as mp,
                  tc.tile_pool(name="z", bufs=3) as zp,
                  tc.tile_pool(name="ups", bufs=2, space="PSUM") as up,
                  tc.tile_pool(name="dps", bufs=2, space="PSUM") as dp,
                  tc.tile_pool(name="tps", bufs=2, space="PSUM") as tp,
                  tc.tile_pool(name="ops", bufs=2, space="PSUM") as op):
                for pair in range(NPAIR):
                    sidx = stp.tile([128, (NLO2 + NHI2) // 16], I16, tag="si")
                    nc.sync.dma_start(sidx[:], sidx_t.ap()[pair])
                    glo = gp.tile([128, NLO2 // BLK, IN_DIM], F16, tag="glo")
                    ghi = gp.tile([128, NHI2 // BLK, IN_DIM], F16, tag="ghi")
                    lob_p = cfg.lob_sb[2 * pair]
                    assert lob_p == cfg.lob_sb[2 * pair + 1]
                    nlo2 = 2 * 4 * lob_p * BLK
                    if pair < NPAIR - 1:
                        nc.gpsimd.dma_gather(glo[:, :nlo2 // BLK, :],
                                             xtab_t.ap()[0:SPLIT, :],
                                             sidx[:, :nlo2 // 16], nlo2, nlo2,
                                             IN_DIM, single_packet=False)
                        nc.gpsimd.dma_gather(ghi[:], xtab_t.ap()[SPLIT:NPAD, :],
                                             sidx[:, nlo2 // 16:
                                                  (nlo2 + NHI2) // 16],
                                             NHI2, NHI2,
                                             IN_DIM, single_packet=False)
                    else:
                        # split the tail pair per-superblock so the last
                        # compute chain overlaps the final gathers
                        nlo16, nhi16 = nlo2 // 32, NHI2 // 32
                        for s2 in range(2):
                            nc.gpsimd.dma_gather(
                                glo[:, s2 * (nlo2 // 256):nlo2 // BLK, :]
                                if s2 else glo[:, :nlo2 // 256, :],
                                xtab_t.ap()[0:SPLIT, :],
                                sidx[:, s2 * nlo16:(s2 + 1) * nlo16],
                                nlo2 // 2, nlo2 // 2, IN_DIM,
                                single_packet=False)
                            nc.gpsimd.dma_gather(
                                ghi[:, s2 * (NHI2 // 256):, :]
                                if s2 else ghi[:, :NHI2 // 256, :],
                                xtab_t.ap()[SPLIT:NPAD, :],
                                sidx[:, nlo2 // 16 + s2 * nhi16:
                                     nlo2 // 16 + (s2 + 1) * nhi16],
                                NHI2 // 2, NHI2 // 2, IN_DIM,
                                single_packet=False)
                    casd = stp.tile([128, 2, 1 + 2 * HEADS, nblk], F16,
                                    tag="casd")
                    nc.sync.dma_start(
                        casd[:].rearrange("p s e b -> p (s e b)"),
                        casd_t.ap()[pair].rearrange("p s e b -> p (s e b)"))
                    otp = zp.tile([128, 2, 128], F16, tag="otp")
                    for s in range(2):
                        sb = 2 * pair + s
                        cx = casd[:, s, 0, :]
                        asd = casd[:, s, 1:1 + 2 * HEADS, :]

                        asum = ap.tile([128, HEADS, nblk], F16, tag="asum")
                        nc.vector.tensor_tensor(out=asum[:],
                                                in0=asd[:, 0:HEADS, :],
                                                in1=asd[:, HEADS:2 * HEADS, :],
                                                op=mybir.AluOpType.add)
                        alr = ap.tile([128, HEADS, nblk], F16, tag="alr")
                        nc.scalar.activation(alr[:], asum[:],
                                             mybir.ActivationFunctionType.Prelu,
                                             alpha=LEAKY_SLOPE)
                        expa = ap.tile([128, HEADS, nblk], F16, tag="expa")
                        nc.scalar.activation(expa[:], alr[:],
                                             mybir.ActivationFunctionType.Exp)

                        oneh = mp.tile([128, WIN, nblk], F16, tag="oneh")
                        nc.vector.tensor_tensor(
                            out=oneh[:],
                            in0=iota_c[:],
                            in1=cx.unsqueeze(1)
                                .to_broadcast([128, WIN, nblk]),
                            op=mybir.AluOpType.is_equal)
                        mw = mp.tile([128, HEADS, WIN, nblk], F16, tag="mw")
                        nc.vector.tensor_tensor(
                            out=mw[:],
                            in0=oneh[:].unsqueeze(1)
                                .to_broadcast([128, HEADS, WIN, nblk]),
                            in1=expa[:].unsqueeze(2)
                                .to_broadcast([128, HEADS, WIN, nblk]),
                            op=mybir.AluOpType.mult)

                        zsb = zp.tile([128, 128], F32, tag="zsb")
                        dsb = zp.tile([128, HEADS], F32, tag="dsb")
                        nl4p = 4 * lob_p
                        for g in range(4):
                            U = up.tile([128, 128], F32, tag="U")
                            dn = dp.tile([WIN, HEADS], F32, tag="dn")
                            cols = ([g * lob_p + j for j in range(lob_p)]
                                    + [nl4p + g * hib + j for j in range(hib)])
                            for j, col in enumerate(cols):
                                if col < nl4p:
                                    rows = glo[:, s * nl4p + col, :]
                                else:
                                    rows = ghi[:, s * 4 * hib + (col - nl4p), :]
                                nc.tensor.matmul(U[:], lhsT=mw[:, :, :, col],
                                                 rhs=rows, start=(j == 0),
                                                 stop=(j == len(cols) - 1))
                            for j, col in enumerate(cols):
                                nc.tensor.matmul(dn[:], lhsT=oneh[:, :, col],
                                                 rhs=expa[:, :, col],
                                                 start=(j == 0),
                                                 stop=(j == len(cols) - 1))
                            usb = zp.tile([128, 128], F16, tag="usb")
                            nc.scalar.copy(usb[:], U[:])
                            ut_ps = tp.tile([128, 128], F16, tag="utp")
                            nc.tensor.transpose(out=ut_ps[:], in_=usb[:],
                                                identity=idp_c[:])
                            ut = zp.tile([128, 128], F16, tag="ut")
                            nc.vector.tensor_copy(ut[:], ut_ps[:])
                            o2 = op.tile([WIN, 128], F32, tag="o2")
                            for h in range(HEADS):
                                sl = slice(h * OUT_DIM, (h + 1) * OUT_DIM)
                                nc.tensor.matmul(o2[:, sl], lhsT=ut[:, sl],
                                                 rhs=w_c[:, sl],
                                                 start=True, stop=True)
                            nc.scalar.copy(zsb[g * WIN:(g + 1) * WIN, :],
                                           o2[:])
                            nc.scalar.copy(dsb[g * WIN:(g + 1) * WIN, :],
                                           dn[:])

                        rec = zp.tile([128, HEADS], F32, tag="rec")
                        nc.vector.reciprocal(rec[:], dsb[:])
                        zt = zp.tile([128, HEADS, OUT_DIM], F16, tag="zt")
                        nc.vector.tensor_tensor(
                            out=zt[:],
                            in0=zsb[:].rearrange("p (h d) -> p h d", h=HEADS),
                            in1=rec[:].unsqueeze(2)
                                .to_broadcast([128, HEADS, OUT_DIM]),
                            op=mybir.AluOpType.mult)
                        ztf = zt[:].rearrange("p h d -> p (h d)")
                        if cfg.bias_nonzero:
                            zt2 = zp.tile([128, 128], F16, tag="zt2")
                            nc.vector.tensor_tensor(out=zt2[:], in0=ztf,
                                                    in1=biasb_c[:],
                                                    op=mybir.AluOpType.add)
                            ztf = zt2[:]
                        # sigmoid via exp (stays in the exp act-table set):
                        # sg = 1/(1+exp(-z))
                        en = zp.tile([128, 128], F16, tag="en")
                        nc.scalar.activation(
                            en[:], ztf, mybir.ActivationFunctionType.Exp,
                            scale=-1.0)
                        den = zp.tile([128, 128], F32, tag="den")
                        nc.vector.tensor_scalar(den[:], en[:], 1.0, None,
                                                mybir.AluOpType.add)
                        sg = zp.tile([128, 128], F32, tag="sg")
                        nc.vector.reciprocal(sg[:], den[:])
                        mix = zp.tile([128, 128], F16, tag="mix")
                        nc.vector.tensor_scalar(mix[:], sg[:], CMIX - BETA,
                                                BETA, mybir.AluOpType.mult,
                                                mybir.AluOpType.add)
                        nc.vector.tensor_tensor(out=otp[:, s, :], in0=ztf,
                                                in1=mix[:],
                                                op=mybir.AluOpType.mult)
                        if pair == NPAIR - 1:
                            nc.sync.dma_start(
                                out_t.ap()[256 * pair:256 * pair + 256, :]
                                    .rearrange("(r s) f -> r s f", s=2)
                                    [:, s, :],
                                otp[:, s, :])
                    if pair < NPAIR - 1:
                        nc.sync.dma_start(
                            out_t.ap()[256 * pair:256 * pair + 256, :]
                                .rearrange("(r s) f -> r (s f)", s=2),
                            otp[:].rearrange("p s f -> p (s f)"))
    nc.compile()
    return nc


# ---------------------------------------------------------------- the API

def run(x, edge_index, W, att_src, att_dst, bias, n_cores=N_CORES,
        trace=False, trace_dir=None):
    x = np.asarray(x, dtype=np.float32)
    W32 = np.asarray(W, dtype=np.float32)
    att_src = np.asarray(att_src, dtype=np.float32)
    att_dst = np.asarray(att_dst, dtype=np.float32)
    bias = np.asarray(bias, dtype=np.float32)
    H, D = att_src.shape

    cfg, per_core, new_row = preprocess(edge_index, n_cores)
    cfg.bias_nonzero = bool(np.any(bias))

    # host-side param-only math + layout casts
    as4 = np.zeros((H * D, 2 * H), dtype=np.float32)
    for h in range(H):
        as4[h * D:(h + 1) * D, h] = att_src[h]
        as4[h * D:(h + 1) * D, H + h] = att_dst[h]
    wad8 = (W32 @ as4).astype(np.float16)
    xtab = np.zeros((NPAD, IN_DIM), dtype=np.float16)
    xtab[:N_NODES] = x.astype(np.float16)
    xT = np.ascontiguousarray(xtab.T)                  # [128, NPAD] fp16
    ident = np.eye(128, dtype=np.float16)
    # iota2[p, c, b] = c  (pre-expanded so the one-hot TT has packed operands)
    iota = np.broadcast_to(
        np.arange(WIN, dtype=np.float16)[None, :, None],
        (128, WIN, cfg.nblk)).copy()
    biasb = np.tile(bias, (128, 1)).astype(np.float32)

    tkw = {}
    tmp1 = tmp2 = None
    if trace:
        tkw = dict(trace=True, trace_cores=list(range(n_cores)))
        if trace_dir:
            tmp1 = os.path.join(trace_dir, "l1")
            tmp2 = os.path.join(trace_dir, "l2")
            os.makedirs(tmp1, exist_ok=True)
            os.makedirs(tmp2, exist_ok=True)

    # launch 1: per-node a_src/a_dst
    npc0 = N_NODES // n_cores
    nc1 = build_nc1(n_cores)
    in_maps1 = []
    for c in range(n_cores):
        slab = np.zeros((128, 6272), dtype=np.float16)
        slab[:, :npc0] = xT[:, c * npc0:(c + 1) * npc0]
        in_maps1.append(dict(xT_slab=slab, wad8=wad8))
    res1 = run_bass_kernel_spmd(nc1, in_maps1, core_ids=list(range(n_cores)),
                                tmpdir=tmp1, **tkw)
    asd8 = np.concatenate(
        [res1.results[c]["adstv"].T[:npc0] for c in range(n_cores)], axis=0)

    # host expansion of per-edge a_src/a_dst (indexing only)
    nc2 = build_nc2(cfg)
    in_maps = []
    for c in range(n_cores):
        pc = per_core[c]
        asd_pe = np.concatenate(
            [asd8[pc["sn"], 0:H], asd8[pc["dn"], H:2 * H]],
            axis=-1).transpose(0, 1, 3, 2)         # [NSB, 128, 8, nblk]
        casd = np.empty((NPAIR, 128, 2, 1 + 2 * H, cfg.nblk),
                        dtype=np.float16)
        casd[:, :, :, 0, :] = pc["cx"].reshape(NPAIR, 2, 128,
                                               cfg.nblk).transpose(0, 2, 1, 3)
        casd[:, :, :, 1:, :] = asd_pe.reshape(
            NPAIR, 2, 128, 2 * H, cfg.nblk).transpose(0, 2, 1, 3, 4)
        in_maps.append(dict(xtab=xtab, w_pd=W32.astype(np.float16),
                            ident_pd=ident, iota_pd=iota, biasb=biasb,
                            sidx=pc["sidx"], casd=casd))
    res = run_bass_kernel_spmd(nc2, in_maps, core_ids=list(range(n_cores)),
                               tmpdir=tmp2, **tkw)
    allout = np.concatenate([res.results[c]["out"] for c in range(n_cores)],
                            axis=0)                    # [51200, 128] fp16
    out = allout[new_row].astype(np.float32)
    parts = dict(nc1=nc1, in_maps1=in_maps1, nc2=nc2, in_maps2=in_maps,
                 res1=res1, res2=res, n_cores=n_cores, cfg=cfg)
    return out, parts


def kernel(**inputs) -> np.ndarray:
    out, _ = run(inputs["x"], inputs["edge_index"], inputs["W"],
                 inputs["att_src"], inputs["att_dst"], inputs["bias"])
    return out


# revision 49
# speedup vs baseline: 1.1128x; 1.0318x over previous
# MixGAT layer (GATConv + beta-mix swish) on 8 Trainium2 NeuronCores.
#
# Strategy (dst-node sharding): nodes are packed into fixed 32-dst "windows"
# (2D bin-packing by lo/hi in-degree so every window fits a static block
# budget); 200 windows per core.  Per superblock (4 windows = 128 dsts):
#   - dma_gather pulls each edge's RAW x row (fp16, 256B) from a host-cast
#     table; aggregation of raw x is exchanged with the W projection
#     (out = (sum_e w_e x[src]) @ W per head).
#   - alpha = lrelu(a_src[src]+a_dst[dst]) -> exp on ACT; a_src/a_dst are
#     per-node values computed on device in launch 1 and expanded per-edge
#     on host (indexing only).
#   - mw[e,(h,c)] = onehot(dst slot) * expa[e,h]; one matmul per 128-edge
#     block accumulates U[(h,c), f] = sum_e mw * x[src]; a second tiny
#     matmul accumulates the softmax denominators dn[c,h].
#   - U is transposed on the PE and projected through W per head, giving
#     node-major z rows; swish postproc and one contiguous output DMA per
#     128 dsts.
#
# kernel(**inputs) is self-contained: preprocessing is pure numpy indexing,
# the device kernels run via run_bass_kernel_spmd on cores 0-7.

import os

import numpy as np

import concourse.bass as bass
import concourse.mybir as mybir
import concourse.tile as tile
from concourse import bacc
from concourse.bass_utils import run_bass_kernel_spmd

F32 = mybir.dt.float32
F16 = mybir.dt.float16
I16 = mybir.dt.int16

# problem constants
N_NODES = 50000
IN_DIM = 128
HEADS = 4
OUT_DIM = 32
LEAKY_SLOPE = 0.2
BETA = 0.5
CMIX = 1.2
N_CORES = 8

# static schedule constants
WIN = 32          # dsts per window / group
BLK = 128         # edges per block (matmul contraction)
WPC = 200         # windows per core
NSB = WPC // 4    # superblocks (128 dsts) per core
NPAIR = NSB // 2  # gather pairs (2 superblocks per gather)
SPLIT = 32768     # int16-addressable table split
NPAD = 50176      # padded table rows (multiple of 128)
DEAD = 100.0      # colidx value for dead slots


class Cfg:
    def __init__(self, lob, hib, bias_nonzero=False, n_cores=N_CORES,
                 n_light=0):
        self.lob = lob                  # max lo blocks per window
        self.hib = hib                  # hi blocks per window
        self.nblk = 4 * (lob + hib)     # block columns per superblock (max)
        self.nlo = 4 * lob * BLK        # lo slots per superblock (max)
        self.nhi = 4 * hib * BLK
        self.bias_nonzero = bias_nonzero
        self.n_cores = n_cores
        self.n_light = n_light  # leading superblocks with one fewer lo block
        self.lob_sb = [lob - 1] * n_light + [lob] * (NSB - n_light)


# ---------------------------------------------------------------- host side

def _wrap16(v):
    """idx vector [S*16] -> dma_gather idx layout [128, S] int16."""
    s = v.reshape(-1, 16).T                      # [16, S]
    return np.tile(s, (8, 1)).astype(np.int16)   # [128, S]


def assign_windows(deg_lo, deg_hi, lo_cap, hi_cap, n_win, lo_caps=None):
    """Deal nodes to windows (hi balanced; lo shaped toward per-window
    targets when lo_caps is given), then repair cap violations.
    Returns win_of[n]. Raises if infeasible under the caps."""
    n = len(deg_lo)
    order = np.argsort(-deg_hi, kind="stable")
    win_of = np.empty(n, dtype=np.int64)
    if lo_caps is None:
        idx = np.arange(n)
        row = idx // n_win
        k = idx % n_win
        w = np.where(row % 2 == 0, k, n_win - 1 - k)
        win_of[order] = w
    else:
        # per-hi-rank rows: give the row's biggest lo-deg nodes to the
        # windows furthest below their (scaled) lo target
        lo_caps = np.asarray(lo_caps, dtype=np.float64)
        total = deg_lo.sum() + (n_win * WIN - n)     # + fakes
        target = lo_caps * (total / lo_caps.sum())
        cur = np.zeros(n_win)
        for k in range((n + n_win - 1) // n_win):
            nodes_k = order[k * n_win:(k + 1) * n_win]
            need_rank = np.argsort(cur - target, kind="stable")  # neediest 1st
            by_lo = nodes_k[np.argsort(-deg_lo[nodes_k], kind="stable")]
            wsel = need_rank[:len(by_lo)]
            win_of[by_lo] = wsel
            cur[wsel] += deg_lo[by_lo]

    cnt = np.bincount(win_of, minlength=n_win)
    assert cnt.max() <= WIN
    # fakes for empty slots count against lo capacity
    losum = np.bincount(win_of, weights=deg_lo, minlength=n_win) + (WIN - cnt)
    hisum = np.bincount(win_of, weights=deg_hi, minlength=n_win)
    locap = (np.full(n_win, lo_cap, dtype=np.float64) if lo_caps is None
             else np.asarray(lo_caps, dtype=np.float64))
    hicap = np.full(n_win, hi_cap, dtype=np.float64)

    # windows -> node lists for swapping
    by_win = [[] for _ in range(n_win)]
    for node in range(n):
        by_win[win_of[node]].append(node)

    def try_fix(sums, caps, other, ocap, deg, odeg):
        for _ in range(20000):
            over = sums - caps
            wbad = int(np.argmax(over))
            if over[wbad] <= 0:
                return True
            # heaviest node in the bad window
            a = max(by_win[wbad], key=lambda x: deg[x])
            done = False
            for wgood in np.argsort(over)[:64]:
                wgood = int(wgood)
                if wgood == wbad:
                    continue
                for b in sorted(by_win[wgood], key=lambda x: deg[x])[:8]:
                    d_s = deg[a] - deg[b]
                    d_o = odeg[a] - odeg[b]
                    if (sums[wbad] - d_s <= caps[wbad]
                            and sums[wgood] + d_s <= caps[wgood]
                            and other[wbad] - d_o <= ocap[wbad]
                            and other[wgood] + d_o <= ocap[wgood]):
                        by_win[wbad].remove(a)
                        by_win[wgood].remove(b)
                        by_win[wbad].append(b)
                        by_win[wgood].append(a)
                        win_of[a], win_of[b] = wgood, wbad
                        sums[wbad] -= d_s
                        sums[wgood] += d_s
                        other[wbad] -= d_o
                        other[wgood] += d_o
                        done = True
                        break
                if done:
                    break
            if not done:
                return False
        return False

    if not try_fix(hisum, hicap, losum, locap, deg_hi, deg_lo):
        raise RuntimeError("hi repair failed")
    if not try_fix(losum, locap, hisum, hicap, deg_lo, deg_hi):
        raise RuntimeError("lo repair failed")
    assert (losum <= locap).all() and (hisum <= hicap).all()
    return win_of


def preprocess(edge_index, n_cores=N_CORES):
    """Window assignment + static per-core gather/colidx/edge-stream arrays.

    Returns (cfg, per_core list, new_row[n] output permutation,
    sn/dn index arrays for the asd expansion)."""
    src0 = np.asarray(edge_index[0], dtype=np.int64)
    dst0 = np.asarray(edge_index[1], dtype=np.int64)
    loop = np.arange(N_NODES, dtype=np.int64)
    src = np.concatenate([src0, loop])
    dst = np.concatenate([dst0, loop])
    lo_mask_e = src < SPLIT
    deg_lo = np.bincount(dst[lo_mask_e], minlength=N_NODES)
    deg_hi = np.bincount(dst[~lo_mask_e], minlength=N_NODES)

    n_win = n_cores * WPC
    cfg = None
    win_of = None
    for lob, hib, n_light in ((6, 3, 32), (6, 3, 28), (6, 3, 24),
                              (6, 3, 0), (7, 3, 0), (7, 4, 0), (8, 4, 0)):
        lo_caps = None
        if n_light:
            # first n_light superblocks per core have lob-1 lo blocks
            wl = np.arange(n_win) % WPC
            lo_caps = np.where(wl // 4 < n_light, (lob - 1) * BLK,
                               lob * BLK).astype(np.float64)
        try:
            win_of = assign_windows(deg_lo, deg_hi, lob * BLK, hib * BLK,
                                    n_win, lo_caps=lo_caps)
            cfg = Cfg(lob, hib, n_light=n_light)
            break
        except RuntimeError:
            continue
    assert cfg is not None, "window packing failed at all cap levels"
    lob, hib = cfg.lob, cfg.hib

    # slot assignment within windows (order of appearance)
    order = np.argsort(win_of, kind="stable")
    slot = np.empty(N_NODES, dtype=np.int64)
    bounds = np.searchsorted(win_of[order], np.arange(n_win + 1))
    for w in range(n_win):
        seg = order[bounds[w]:bounds[w + 1]]
        slot[seg] = np.arange(len(seg))
    # output row of each original node; the per-pair output DMA interleaves
    # the two superblocks of a pair: row = 256*pair + 2*(g*32+slot) + s
    core_of = win_of // WPC
    wl = win_of % WPC
    sb_of, g_of = wl // 4, wl % 4
    new_row = (core_of * (WPC * WIN) + (sb_of // 2) * 256
               + 2 * (g_of * WIN + slot) + (sb_of % 2))

    # per-edge: window / slot of the dst
    e_win = win_of[dst]
    e_slot = slot[dst]
    e_order = np.argsort(e_win, kind="stable")
    eb = np.searchsorted(e_win[e_order], np.arange(n_win + 1))

    per_core = []
    for c in range(n_cores):
        ilo = np.zeros((NSB, 4, lob, BLK), dtype=np.int64)
        ihi = np.zeros((NSB, 4, hib, BLK), dtype=np.int64)
        clo = np.full((NSB, 4, lob, BLK), DEAD, dtype=np.float16)
        chi = np.full((NSB, 4, hib, BLK), DEAD, dtype=np.float16)
        snlo = np.zeros((NSB, 4, lob, BLK), dtype=np.int64)
        dnlo = np.zeros((NSB, 4, lob, BLK), dtype=np.int64)
        snhi = np.zeros((NSB, 4, hib, BLK), dtype=np.int64)
        dnhi = np.zeros((NSB, 4, hib, BLK), dtype=np.int64)
        for wl in range(WPC):
            w = c * WPC + wl
            sb, g = wl // 4, wl % 4
            lob_s = cfg.lob_sb[sb]
            seg = e_order[eb[w]:eb[w + 1]]
            es, ec, ed = src[seg], e_slot[seg], dst[seg]
            m = es < SPLIT
            # lo side, with fake self-edges for empty slots
            nfake = WIN - (bounds[w + 1] - bounds[w])
            ls = np.concatenate([es[m], np.zeros(nfake, dtype=np.int64)])
            lc = np.concatenate([ec[m],
                                 np.arange(WIN - nfake, WIN, dtype=np.int64)])
            ld = np.concatenate([ed[m], np.zeros(nfake, dtype=np.int64)])
            nl = len(ls)
            assert nl <= lob_s * BLK, (w, nl)
            ilo[sb, g].reshape(-1)[:nl] = ls
            clo[sb, g].reshape(-1)[:nl] = lc.astype(np.float16)
            snlo[sb, g].reshape(-1)[:nl] = ls
            dnlo[sb, g].reshape(-1)[:nl] = ld
            hs, hc, hd = es[~m], ec[~m], ed[~m]
            nh = len(hs)
            assert nh <= hib * BLK, (w, nh)
            ihi[sb, g].reshape(-1)[:nh] = hs - SPLIT
            chi[sb, g].reshape(-1)[:nh] = hc.astype(np.float16)
            snhi[sb, g].reshape(-1)[:nh] = hs
            dnhi[sb, g].reshape(-1)[:nh] = hd

        # gather idx per pair: [NPAIR, 128, (2*nlo + 2*nhi)/16] (zero pad)
        sidx = np.zeros((NPAIR, 128, (2 * cfg.nlo + 2 * cfg.nhi) // 16),
                        dtype=np.int16)
        for p in range(NPAIR):
            lob_p = cfg.lob_sb[2 * p]
            vlo = ilo[2 * p:2 * p + 2, :, :lob_p, :].reshape(-1)
            vhi = ihi[2 * p:2 * p + 2].reshape(-1)
            nc_lo = len(vlo) // 16
            sidx[p, :, :nc_lo] = _wrap16(vlo)
            sidx[p, :, nc_lo:nc_lo + len(vhi) // 16] = _wrap16(vhi)

        # colidx [NSB, 128, NBLK] (block-col major: 4*lob_s lo | 4*hib hi)
        cx = np.full((NSB, 128, cfg.nblk), DEAD, dtype=np.float16)
        sn = np.zeros((NSB, 128, cfg.nblk), dtype=np.int64)
        dn = np.zeros((NSB, 128, cfg.nblk), dtype=np.int64)
        for sb in range(NSB):
            lob_s = cfg.lob_sb[sb]
            nl4 = 4 * lob_s
            nh4 = 4 * hib
            cx[sb, :, :nl4] = clo[sb, :, :lob_s].reshape(nl4, BLK).T
            cx[sb, :, nl4:nl4 + nh4] = chi[sb].reshape(nh4, BLK).T
            sn[sb, :, :nl4] = snlo[sb, :, :lob_s].reshape(nl4, BLK).T
            sn[sb, :, nl4:nl4 + nh4] = snhi[sb].reshape(nh4, BLK).T
            dn[sb, :, :nl4] = dnlo[sb, :, :lob_s].reshape(nl4, BLK).T
            dn[sb, :, nl4:nl4 + nh4] = dnhi[sb].reshape(nh4, BLK).T
        per_core.append(dict(sidx=sidx, cx=cx, sn=sn, dn=dn))
    return cfg, per_core, new_row


# -------------------------------------------------------------- device side

def build_nc1(n_cores=N_CORES):
    """Launch 1: asd [104, 512] = per-node (a_src[4], a_dst[4]) for the
    core's 6250-node slab (padded to 6272), from xT fp16 and wad8."""
    nc = bacc.Bacc("TRN2", target_bir_lowering=False, debug=False,
                   num_devices=n_cores)
    xs_t = nc.dram_tensor("xT_slab", [128, 6272], F16, kind="ExternalInput")
    wad_t = nc.dram_tensor("wad8", [IN_DIM, 2 * HEADS], F16,
                           kind="ExternalInput")
    out_t = nc.dram_tensor("adstv", [2 * HEADS, 6272], F32,
                           kind="ExternalOutput")
    with tile.TileContext(nc) as tc:
        with (tc.tile_pool(name="c", bufs=1) as cp,
              tc.tile_pool(name="p", bufs=2, space="PSUM") as pp):
            wad_c = cp.tile([IN_DIM, 2 * HEADS], F16)
            nc.sync.dma_start(wad_c[:], wad_t.ap())
            xc = cp.tile([128, 6272], F16)
            for k in range(13):
                n0 = 512 * k
                nk = min(512, 6272 - n0)
                nc.sync.dma_start(xc[:, n0:n0 + nk], xs_t.ap()[:, n0:n0 + nk])
            acc = cp.tile([2 * HEADS, 6272], F32)
            for k in range(13):
                n0 = 512 * k
                nk = min(512, 6272 - n0)
                ps = pp.tile([2 * HEADS, 512], F32, tag="ps")
                nc.tensor.matmul(ps[:, :nk], lhsT=wad_c[:],
                                 rhs=xc[:, n0:n0 + nk], start=True, stop=True)
                if k % 2 == 0:
                    nc.scalar.copy(acc[:, n0:n0 + nk], ps[:, :nk])
                else:
                    nc.vector.tensor_copy(acc[:, n0:n0 + nk], ps[:, :nk])
            nc.sync.dma_start(out_t.ap(), acc[:])
    nc.compile()
    return nc


def build_nc2(cfg: Cfg):
    nc = bacc.Bacc("TRN2", target_bir_lowering=False, debug=False,
                   num_devices=cfg.n_cores)
    lob, hib, nblk = cfg.lob, cfg.hib, cfg.nblk
    nl4 = 4 * lob
    NLO2, NHI2 = 2 * cfg.nlo, 2 * cfg.nhi

    xtab_t = nc.dram_tensor("xtab", [NPAD, IN_DIM], F16, kind="ExternalInput")
    w_t = nc.dram_tensor("w_pd", [IN_DIM, HEADS * OUT_DIM], F16,
                         kind="ExternalInput")
    idp_t = nc.dram_tensor("ident_pd", [128, 128], F16, kind="ExternalInput")
    iota_t = nc.dram_tensor("iota_pd", [128, WIN, nblk], F16,
                            kind="ExternalInput")
    biasb_t = nc.dram_tensor("biasb", [128, HEADS * OUT_DIM], F32,
                             kind="ExternalInput")
    sidx_t = nc.dram_tensor("sidx", [NPAIR, 128, (NLO2 + NHI2) // 16], I16,
                            kind="ExternalInput")
    casd_t = nc.dram_tensor("casd", [NPAIR, 128, 2, 1 + 2 * HEADS, nblk],
                            F16, kind="ExternalInput")
    out_t = nc.dram_tensor("out", [WPC * WIN, HEADS * OUT_DIM], F16,
                           kind="ExternalOutput")

    with tile.TileContext(nc) as tc:
        with tc.tile_pool(name="consts", bufs=1) as cpool:
            w_c = cpool.tile([IN_DIM, HEADS * OUT_DIM], F16)
            nc.sync.dma_start(w_c[:], w_t.ap())
            idp_c = cpool.tile([128, 128], F16)
            nc.sync.dma_start(idp_c[:], idp_t.ap())
            iota_c = cpool.tile([128, WIN, nblk], F16)
            nc.sync.dma_start(iota_c[:].rearrange("p c b -> p (c b)"),
                                iota_t.ap().rearrange("p c b -> p (c b)"))
            biasb_c = cpool.tile([128, HEADS * OUT_DIM], F32)
            nc.sync.dma_start(biasb_c[:], biasb_t.ap())

            with (tc.tile_pool(name="st", bufs=4) as stp,
                  tc.tile_pool(name="g", bufs=4) as gp,
                  tc.tile_pool(name="al", bufs=3) as ap,
                  tc.tile_pool(name="m", bufs=4) as mp,
                  tc.tile_pool(name="z", bufs=3) as zp,
                  tc.tile_pool(name="ups", bufs=2, space="PSUM") as up,
                  tc.tile_pool(name="dps", bufs=2, space="PSUM") as dp,
                  tc.tile_pool(name="tps", bufs=2, space="PSUM") as tp,
                  tc.tile_pool(name="ops", bufs=2, space="PSUM") as op):
                for pair in range(NPAIR):
                    sidx = stp.tile([128, (NLO2 + NHI2) // 16], I16, tag="si")
                    nc.sync.dma_start(sidx[:], sidx_t.ap()[pair])
                    glo = gp.tile([128, NLO2 // BLK, IN_DIM], F16, tag="glo")
                    ghi = gp.tile([128, NHI2 // BLK, IN_DIM], F16, tag="ghi")
                    lob_p = cfg.lob_sb[2 * pair]
                    assert lob_p == cfg.lob_sb[2 * pair + 1]
                    nlo2 = 2 * 4 * lob_p * BLK
                    if pair < NPAIR - 1:
                        nc.gpsimd.dma_gather(glo[:, :nlo2 // BLK, :],
                                             xtab_t.ap()[0:SPLIT, :],
                                             sidx[:, :nlo2 // 16], nlo2, nlo2,
                                             IN_DIM, single_packet=False)
                        nc.gpsimd.dma_gather(ghi[:], xtab_t.ap()[SPLIT:NPAD, :],
                                             sidx[:, nlo2 // 16:
                                                  (nlo2 + NHI2) // 16],
                                             NHI2, NHI2,
                                             IN_DIM, single_packet=False)
                    else:
                        # split the tail pair per-superblock so the last
                        # compute chain overlaps the final gathers
                        nlo16, nhi16 = nlo2 // 32, NHI2 // 32
                        for s2 in range(2):
                            nc.gpsimd.dma_gather(
                                glo[:, s2 * (nlo2 // 256):nlo2 // BLK, :]
                                if s2 else glo[:, :nlo2 // 256, :],
                                xtab_t.ap()[0:SPLIT, :],
                                sidx[:, s2 * nlo16:(s2 + 1) * nlo16],
                                nlo2 // 2, nlo2 // 2, IN_DIM,
                                single_packet=False)
                            nc.gpsimd.dma_gather(
                                ghi[:, s2 * (NHI2 // 256):, :]
                                if s2 else ghi[:, :NHI2 // 256, :],
                                xtab_t.ap()[SPLIT:NPAD, :],
                                sidx[:, nlo2 // 16 + s2 * nhi16:
                                     nlo2 // 16 + (s2 + 1) * nhi16],
                                NHI2 // 2, NHI2 // 2, IN_DIM,
                                single_packet=False)
                    casd = stp.tile([128, 2, 1 + 2 * HEADS, nblk], F16,
                                    tag="casd")
                    nc.sync.dma_start(
                        casd[:].rearrange("p s e b -> p (s e b)"),
                        casd_t.ap()[pair].rearrange("p s e b -> p (s e b)"))
                    otp = zp.tile([128, 2, 128], F16, tag="otp")
                    for s in range(2):
                        sb = 2 * pair + s
                        cx = casd[:, s, 0, :]
                        asd = casd[:, s, 1:1 + 2 * HEADS, :]

                        asum = ap.tile([128, HEADS, nblk], F16, tag="asum")
                        nc.vector.tensor_tensor(out=asum[:],
                                                in0=asd[:, 0:HEADS, :],
                                                in1=asd[:, HEADS:2 * HEADS, :],
                                                op=mybir.AluOpType.add)
                        alr = ap.tile([128, HEADS, nblk], F16, tag="alr")
                        nc.scalar.activation(alr[:], asum[:],
                                             mybir.ActivationFunctionType.Prelu,
                                             alpha=LEAKY_SLOPE)
                        expa = ap.tile([128, HEADS, nblk], F16, tag="expa")
                        nc.scalar.activation(expa[:], alr[:],
                                             mybir.ActivationFunctionType.Exp)

                        oneh = mp.tile([128, WIN, nblk], F16, tag="oneh")
                        nc.vector.tensor_tensor(
                            out=oneh[:],
                            in0=iota_c[:],
                            in1=cx.unsqueeze(1)
                                .to_broadcast([128, WIN, nblk]),
                            op=mybir.AluOpType.is_equal)
                        mw = mp.tile([128, HEADS, WIN, nblk], F16, tag="mw")
                        nc.vector.tensor_tensor(
                            out=mw[:],
                            in0=oneh[:].unsqueeze(1)
                                .to_broadcast([128, HEADS, WIN, nblk]),
                            in1=expa[:].unsqueeze(2)
                                .to_broadcast([128, HEADS, WIN, nblk]),
                            op=mybir.AluOpType.mult)

                        zsb = zp.tile([128, 128], F32, tag="zsb")
                        dsb = zp.tile([128, HEADS], F32, tag="dsb")
                        nl4p = 4 * lob_p
                        for g in range(4):
                            U = up.tile([128, 128], F32, tag="U")
                            dn = dp.tile([WIN, HEADS], F32, tag="dn")
                            cols = ([g * lob_p + j for j in range(lob_p)]
                                    + [nl4p + g * hib + j for j in range(hib)])
                            for j, col in enumerate(cols):
                                if col < nl4p:
                                    rows = glo[:, s * nl4p + col, :]
                                else:
                                    rows = ghi[:, s * 4 * hib + (col - nl4p), :]
                                nc.tensor.matmul(U[:], lhsT=mw[:, :, :, col],
                                                 rhs=rows, start=(j == 0),
                                                 stop=(j == len(cols) - 1))
                            for j, col in enumerate(cols):
                                nc.tensor.matmul(dn[:], lhsT=oneh[:, :, col],
                                                 rhs=expa[:, :, col],
                                                 start=(j == 0),
                                                 stop=(j == len(cols) - 1))
                            usb = zp.tile([128, 128], F16, tag="usb")
                            nc.scalar.copy(usb[:], U[:])
                            ut_ps = tp.tile([128, 128], F16, tag="utp")
                            nc.tensor.transpose(out=ut_ps[:], in_=usb[:],
                                                identity=idp_c[:])
                            ut = zp.tile([128, 128], F16, tag="ut")
                            nc.vector.tensor_copy(ut[:], ut_ps[:])
                            o2 = op.tile([WIN, 128], F32, tag="o2")
                            for h in range(HEADS):
                                sl = slice(h * OUT_DIM, (h + 1) * OUT_DIM)
                                nc.tensor.matmul(o2[:, sl], lhsT=ut[:, sl],
                                                 rhs=w_c[:, sl],
                                                 start=True, stop=True)
                            nc.scalar.copy(zsb[g * WIN:(g + 1) * WIN, :],
                                           o2[:])
                            nc.scalar.copy(dsb[g * WIN:(g + 1) * WIN, :],
                                           dn[:])

                        rec = zp.tile([128, HEADS], F32, tag="rec")
                        nc.vector.reciprocal(rec[:], dsb[:])
                        zt = zp.tile([128, HEADS, OUT_DIM], F16, tag="zt")
                        nc.vector.tensor_tensor(
                            out=zt[:],
                            in0=zsb[:].rearrange("p (h d) -> p h d", h=HEADS),
                            in1=rec[:].unsqueeze(2)
                                .to_broadcast([128, HEADS, OUT_DIM]),
                            op=mybir.AluOpType.mult)
                        ztf = zt[:].rearrange("p h d -> p (h d)")
                        if cfg.bias_nonzero:
                            zt2 = zp.tile([128, 128], F16, tag="zt2")
                            nc.vector.tensor_tensor(out=zt2[:], in0=ztf,
                                                    in1=biasb_c[:],
                                                    op=mybir.AluOpType.add)
                            ztf = zt2[:]
                        # sigmoid via exp (stays in the exp act-table set):
                        # sg = 1/(1+exp(-z))
                        en = zp.tile([128, 128], F16, tag="en")
                        nc.scalar.activation(
                            en[:], ztf, mybir.ActivationFunctionType.Exp,
                            scale=-1.0)
                        den = zp.tile([128, 128], F32, tag="den")
                        nc.vector.tensor_scalar(den[:], en[:], 1.0, None,
                                                mybir.AluOpType.add)
                        sg = zp.tile([128, 128], F32, tag="sg")
                        nc.vector.reciprocal(sg[:], den[:])
                        mix = zp.tile([128, 128], F16, tag="mix")
                        nc.vector.tensor_scalar(mix[:], sg[:], CMIX - BETA,
                                                BETA, mybir.AluOpType.mult,
                                                mybir.AluOpType.add)
                        nc.vector.tensor_tensor(out=otp[:, s, :], in0=ztf,
                                                in1=mix[:],
                                                op=mybir.AluOpType.mult)
                        if pair == NPAIR - 1:
                            nc.sync.dma_start(
                                out_t.ap()[256 * pair:256 * pair + 256, :]
                                    .rearrange("(r s) f -> r s f", s=2)
                                    [:, s, :],
                                otp[:, s, :])
                    if pair < NPAIR - 1:
                        nc.sync.dma_start(
                            out_t.ap()[256 * pair:256 * pair + 256, :]
                                .rearrange("(r s) f -> r (s f)", s=2),
                            otp[:].rearrange("p s f -> p (s f)"))
    nc.compile()
    return nc


# ---------------------------------------------------------------- the API

def run(x, edge_index, W, att_src, att_dst, bias, n_cores=N_CORES,
        trace=False, trace_dir=None):
    x = np.asarray(x, dtype=np.float32)
    W32 = np.asarray(W, dtype=np.float32)
    att_src = np.asarray(att_src, dtype=np.float32)
    att_dst = np.asarray(att_dst, dtype=np.float32)
    bias = np.asarray(bias, dtype=np.float32)
    H, D = att_src.shape

    cfg, per_core, new_row = preprocess(edge_index, n_cores)
    cfg.bias_nonzero = bool(np.any(bias))

    # host-side param-only math + layout casts
    as4 = np.zeros((H * D, 2 * H), dtype=np.float32)
    for h in range(H):
        as4[h * D:(h + 1) * D, h] = att_src[h]
        as4[h * D:(h + 1) * D, H + h] = att_dst[h]
    wad8 = (W32 @ as4).astype(np.float16)
    xtab = np.zeros((NPAD, IN_DIM), dtype=np.float16)
    xtab[:N_NODES] = x.astype(np.float16)
    xT = np.ascontiguousarray(xtab.T)                  # [128, NPAD] fp16
    ident = np.eye(128, dtype=np.float16)
    # iota2[p, c, b] = c  (pre-expanded so the one-hot TT has packed operands)
    iota = np.broadcast_to(
        np.arange(WIN, dtype=np.float16)[None, :, None],
        (128, WIN, cfg.nblk)).copy()
    biasb = np.tile(bias, (128, 1)).astype(np.float32)

    tkw = {}
    tmp1 = tmp2 = None
    if trace:
        tkw = dict(trace=True, trace_cores=list(range(n_cores)))
        if trace_dir:
            tmp1 = os.path.join(trace_dir, "l1")
            tmp2 = os.path.join(trace_dir, "l2")
            os.makedirs(tmp1, exist_ok=True)
            os.makedirs(tmp2, exist_ok=True)

    # launch 1: per-node a_src/a_dst
    npc0 = N_NODES // n_cores
    nc1 = build_nc1(n_cores)
    in_maps1 = []
    for c in range(n_cores):
        slab = np.zeros((128, 6272), dtype=np.float16)
        slab[:, :npc0] = xT[:, c * npc0:(c + 1) * npc0]
        in_maps1.append(dict(xT_slab=slab, wad8=wad8))
    res1 = run_bass_kernel_spmd(nc1, in_maps1, core_ids=list(range(n_cores)),
                                tmpdir=tmp1, **tkw)
    asd8 = np.concatenate(
        [res1.results[c]["adstv"].T[:npc0] for c in range(n_cores)], axis=0)

    # host expansion of per-edge a_src/a_dst (indexing only)
    nc2 = build_nc2(cfg)
    in_maps = []
    for c in range(n_cores):
        pc = per_core[c]
        asd_pe = np.concatenate(
            [asd8[pc["sn"], 0:H], asd8[pc["dn"], H:2 * H]],
            axis=-1).transpose(0, 1, 3, 2)         # [NSB, 128, 8, nblk]
        casd = np.empty((NPAIR, 128, 2, 1 + 2 * H, cfg.nblk),
                        dtype=np.float16)
        casd[:, :, :, 0, :] = pc["cx"].reshape(NPAIR, 2, 128,
                                               cfg.nblk).transpose(0, 2, 1, 3)
        casd[:, :, :, 1:, :] = asd_pe.reshape(
            NPAIR, 2, 128, 2 * H, cfg.nblk).transpose(0, 2, 1, 3, 4)
        in_maps.append(dict(xtab=xtab, w_pd=W32.astype(np.float16),
                            ident_pd=ident, iota_pd=iota, biasb=biasb,
                            sidx=pc["sidx"], casd=casd))
    res = run_bass_kernel_spmd(nc2, in_maps, core_ids=list(range(n_cores)),
                               tmpdir=tmp2, **tkw)
    allout = np.concatenate([res.results[c]["out"] for c in range(n_cores)],
                            axis=0)                    # [51200, 128] fp16
    out = allout[new_row].astype(np.float32)
    parts = dict(nc1=nc1, in_maps1=in_maps1, nc2=nc2, in_maps2=in_maps,
                 res1=res1, res2=res, n_cores=n_cores, cfg=cfg)
    return out, parts


def kernel(**inputs) -> np.ndarray:
    out, _ = run(inputs["x"], inputs["edge_index"], inputs["W"],
                 inputs["att_src"], inputs["att_dst"], inputs["bias"])
    return out


# revision 64
# speedup vs baseline: 1.1300x; 1.0155x over previous
# MixGAT layer (GATConv + beta-mix swish) on 8 Trainium2 NeuronCores.
#
# Strategy (dst-node sharding): nodes are packed into fixed 32-dst "windows"
# (2D bin-packing by lo/hi in-degree so every window fits a static per-window
# block budget; the budgets themselves are two-tier - the first n_light
# superblocks per core get one fewer lo block, and the packer shapes window
# degree sums to match - which trims ~10% of gather descriptors).
# 200 windows per core.  Per superblock (4 windows = 128 dsts):
#   - dma_gather pulls each edge's RAW x row (fp16, 256B) from a host-cast
#     table; aggregation of raw x is exchanged with the W projection
#     (out = (sum_e w_e x[src]) @ W per head), so no projection table is
#     built on device and phase A disappears entirely.
#   - alpha = lrelu(a_src[src]+a_dst[dst]) -> Prelu+Exp on ACT (both live in
#     the same activation-table set as Copy, so no table reloads); a_src /
#     a_dst are per-node values computed on device in launch 1 and expanded
#     per-edge on host (indexing only).  Stream layouts keep the block dim
#     innermost so DVE element-wise ops hit the 2x fp16 perf mode.
#   - mw[e,(h,c)] = onehot(dst slot) * expa[e,h]; one matmul per 128-edge
#     block accumulates U[(h,c), f] = sum_e mw * x[src]; a second tiny
#     matmul accumulates the softmax denominators dn[c,h].
#   - U is transposed on the PE and projected through W per head, giving
#     node-major z rows; swish postproc (sigmoid via exp to stay in one
#     act-table set) and one 512B-per-partition output DMA per pair.
#
# kernel(**inputs) is self-contained: preprocessing is pure numpy indexing,
# the device kernels run via run_bass_kernel_spmd on cores 0-7.

import os

import numpy as np

import concourse.bass as bass
import concourse.mybir as mybir
import concourse.tile as tile
from concourse import bacc
from concourse.bass_utils import run_bass_kernel_spmd

F32 = mybir.dt.float32
F16 = mybir.dt.float16
I16 = mybir.dt.int16

# problem constants
N_NODES = 50000
IN_DIM = 128
HEADS = 4
OUT_DIM = 32
LEAKY_SLOPE = 0.2
BETA = 0.5
CMIX = 1.2
N_CORES = 8

# static schedule constants
WIN = 32          # dsts per window / group
BLK = 128         # edges per block (matmul contraction)
WPC = 200         # windows per core
NSB = WPC // 4    # superblocks (128 dsts) per core
NPAIR = NSB // 2  # gather pairs (2 superblocks per gather)
SPLIT = 32768     # int16-addressable table split
NPAD = 50176      # padded table rows (multiple of 128)
DEAD = 100.0      # colidx value for dead slots


class Cfg:
    def __init__(self, lob, hib, bias_nonzero=False, n_cores=N_CORES,
                 n_light=0):
        self.lob = lob                  # max lo blocks per window
        self.hib = hib                  # hi blocks per window
        self.nblk = 4 * (lob + hib)     # block columns per superblock (max)
        self.nlo = 4 * lob * BLK        # lo slots per superblock (max)
        self.nhi = 4 * hib * BLK
        self.bias_nonzero = bias_nonzero
        self.n_cores = n_cores
        self.n_light = n_light  # leading superblocks with one fewer lo block
        self.lob_sb = [lob - 1] * n_light + [lob] * (NSB - n_light)


# ---------------------------------------------------------------- host side

def _wrap16(v):
    """idx vector [S*16] -> dma_gather idx layout [128, S] int16."""
    s = v.reshape(-1, 16).T                      # [16, S]
    return np.tile(s, (8, 1)).astype(np.int16)   # [128, S]


def assign_windows(deg_lo, deg_hi, lo_cap, hi_cap, n_win, lo_caps=None):
    """Deal nodes to windows (hi balanced; lo shaped toward per-window
    targets when lo_caps is given), then repair cap violations.
    Returns win_of[n]. Raises if infeasible under the caps."""
    n = len(deg_lo)
    order = np.argsort(-deg_hi, kind="stable")
    win_of = np.empty(n, dtype=np.int64)
    if lo_caps is None:
        idx = np.arange(n)
        row = idx // n_win
        k = idx % n_win
        w = np.where(row % 2 == 0, k, n_win - 1 - k)
        win_of[order] = w
    else:
        # per-hi-rank rows: give the row's biggest lo-deg nodes to the
        # windows furthest below their (scaled) lo target
        lo_caps = np.asarray(lo_caps, dtype=np.float64)
        total = deg_lo.sum() + (n_win * WIN - n)     # + fakes
        target = lo_caps * (total / lo_caps.sum())
        cur = np.zeros(n_win)
        for k in range((n + n_win - 1) // n_win):
            nodes_k = order[k * n_win:(k + 1) * n_win]
            need_rank = np.argsort(cur - target, kind="stable")  # neediest 1st
            by_lo = nodes_k[np.argsort(-deg_lo[nodes_k], kind="stable")]
            wsel = need_rank[:len(by_lo)]
            win_of[by_lo] = wsel
            cur[wsel] += deg_lo[by_lo]

    cnt = np.bincount(win_of, minlength=n_win)
    assert cnt.max() <= WIN
    # fakes for empty slots count against lo capacity
    losum = np.bincount(win_of, weights=deg_lo, minlength=n_win) + (WIN - cnt)
    hisum = np.bincount(win_of, weights=deg_hi, minlength=n_win)
    locap = (np.full(n_win, lo_cap, dtype=np.float64) if lo_caps is None
             else np.asarray(lo_caps, dtype=np.float64))
    hicap = np.full(n_win, hi_cap, dtype=np.float64)

    # windows -> node lists for swapping
    by_win = [[] for _ in range(n_win)]
    for node in range(n):
        by_win[win_of[node]].append(node)

    def try_fix(sums, caps, other, ocap, deg, odeg):
        for _ in range(20000):
            over = sums - caps
            wbad = int(np.argmax(over))
            if over[wbad] <= 0:
                return True
            # heaviest node in the bad window
            a = max(by_win[wbad], key=lambda x: deg[x])
            done = False
            for wgood in np.argsort(over)[:64]:
                wgood = int(wgood)
                if wgood == wbad:
                    continue
                for b in sorted(by_win[wgood], key=lambda x: deg[x])[:8]:
                    d_s = deg[a] - deg[b]
                    d_o = odeg[a] - odeg[b]
                    if (sums[wbad] - d_s <= caps[wbad]
                            and sums[wgood] + d_s <= caps[wgood]
                            and other[wbad] - d_o <= ocap[wbad]
                            and other[wgood] + d_o <= ocap[wgood]):
                        by_win[wbad].remove(a)
                        by_win[wgood].remove(b)
                        by_win[wbad].append(b)
                        by_win[wgood].append(a)
                        win_of[a], win_of[b] = wgood, wbad
                        sums[wbad] -= d_s
                        sums[wgood] += d_s
                        other[wbad] -= d_o
                        other[wgood] += d_o
                        done = True
                        break
                if done:
                    break
            if not done:
                return False
        return False

    if not try_fix(hisum, hicap, losum, locap, deg_hi, deg_lo):
        raise RuntimeError("hi repair failed")
    if not try_fix(losum, locap, hisum, hicap, deg_lo, deg_hi):
        raise RuntimeError("lo repair failed")
    assert (losum <= locap).all() and (hisum <= hicap).all()
    return win_of


def preprocess(edge_index, n_cores=N_CORES):
    """Window assignment + static per-core gather/colidx/edge-stream arrays.

    Returns (cfg, per_core list, new_row[n] output permutation,
    sn/dn index arrays for the asd expansion)."""
    src0 = np.asarray(edge_index[0], dtype=np.int64)
    dst0 = np.asarray(edge_index[1], dtype=np.int64)
    loop = np.arange(N_NODES, dtype=np.int64)
    src = np.concatenate([src0, loop])
    dst = np.concatenate([dst0, loop])
    lo_mask_e = src < SPLIT
    deg_lo = np.bincount(dst[lo_mask_e], minlength=N_NODES)
    deg_hi = np.bincount(dst[~lo_mask_e], minlength=N_NODES)

    n_win = n_cores * WPC
    cfg = None
    win_of = None
    for lob, hib, n_light in ((6, 3, 32), (6, 3, 28), (6, 3, 24),
                              (6, 3, 0), (7, 3, 0), (7, 4, 0), (8, 4, 0)):
        lo_caps = None
        if n_light:
            # first n_light superblocks per core have lob-1 lo blocks
            wl = np.arange(n_win) % WPC
            lo_caps = np.where(wl // 4 < n_light, (lob - 1) * BLK,
                               lob * BLK).astype(np.float64)
        try:
            win_of = assign_windows(deg_lo, deg_hi, lob * BLK, hib * BLK,
                                    n_win, lo_caps=lo_caps)
            cfg = Cfg(lob, hib, n_light=n_light)
            break
        except RuntimeError:
            continue
    assert cfg is not None, "window packing failed at all cap levels"
    lob, hib = cfg.lob, cfg.hib

    # slot assignment within windows (order of appearance)
    order = np.argsort(win_of, kind="stable")
    slot = np.empty(N_NODES, dtype=np.int64)
    bounds = np.searchsorted(win_of[order], np.arange(n_win + 1))
    for w in range(n_win):
        seg = order[bounds[w]:bounds[w + 1]]
        slot[seg] = np.arange(len(seg))
    # output row of each original node; the per-pair output DMA interleaves
    # the two superblocks of a pair: row = 256*pair + 2*(g*32+slot) + s
    core_of = win_of // WPC
    wl = win_of % WPC
    sb_of, g_of = wl // 4, wl % 4
    new_row = (core_of * (WPC * WIN) + (sb_of // 2) * 256
               + 2 * (g_of * WIN + slot) + (sb_of % 2))

    # per-edge: window / slot of the dst
    e_win = win_of[dst]
    e_slot = slot[dst]
    e_order = np.argsort(e_win, kind="stable")
    eb = np.searchsorted(e_win[e_order], np.arange(n_win + 1))

    per_core = []
    for c in range(n_cores):
        ilo = np.zeros((NSB, 4, lob, BLK), dtype=np.int64)
        ihi = np.zeros((NSB, 4, hib, BLK), dtype=np.int64)
        clo = np.full((NSB, 4, lob, BLK), DEAD, dtype=np.float16)
        chi = np.full((NSB, 4, hib, BLK), DEAD, dtype=np.float16)
        snlo = np.zeros((NSB, 4, lob, BLK), dtype=np.int64)
        dnlo = np.zeros((NSB, 4, lob, BLK), dtype=np.int64)
        snhi = np.zeros((NSB, 4, hib, BLK), dtype=np.int64)
        dnhi = np.zeros((NSB, 4, hib, BLK), dtype=np.int64)
        for wl in range(WPC):
            w = c * WPC + wl
            sb, g = wl // 4, wl % 4
            lob_s = cfg.lob_sb[sb]
            seg = e_order[eb[w]:eb[w + 1]]
            es, ec, ed = src[seg], e_slot[seg], dst[seg]
            m = es < SPLIT
            # lo side, with fake self-edges for empty slots
            nfake = WIN - (bounds[w + 1] - bounds[w])
            ls = np.concatenate([es[m], np.zeros(nfake, dtype=np.int64)])
            lc = np.concatenate([ec[m],
                                 np.arange(WIN - nfake, WIN, dtype=np.int64)])
            ld = np.concatenate([ed[m], np.zeros(nfake, dtype=np.int64)])
            nl = len(ls)
            assert nl <= lob_s * BLK, (w, nl)
            ilo[sb, g].reshape(-1)[:nl] = ls
            clo[sb, g].reshape(-1)[:nl] = lc.astype(np.float16)
            snlo[sb, g].reshape(-1)[:nl] = ls
            dnlo[sb, g].reshape(-1)[:nl] = ld
            hs, hc, hd = es[~m], ec[~m], ed[~m]
            nh = len(hs)
            assert nh <= hib * BLK, (w, nh)
            ihi[sb, g].reshape(-1)[:nh] = hs - SPLIT
            chi[sb, g].reshape(-1)[:nh] = hc.astype(np.float16)
            snhi[sb, g].reshape(-1)[:nh] = hs
            dnhi[sb, g].reshape(-1)[:nh] = hd

        # gather idx per pair: [NPAIR, 128, (2*nlo + 2*nhi)/16] (zero pad)
        sidx = np.zeros((NPAIR, 128, (2 * cfg.nlo + 2 * cfg.nhi) // 16),
                        dtype=np.int16)
        for p in range(NPAIR):
            lob_p = cfg.lob_sb[2 * p]
            vlo = ilo[2 * p:2 * p + 2, :, :lob_p, :].reshape(-1)
            vhi = ihi[2 * p:2 * p + 2].reshape(-1)
            nc_lo = len(vlo) // 16
            sidx[p, :, :nc_lo] = _wrap16(vlo)
            sidx[p, :, nc_lo:nc_lo + len(vhi) // 16] = _wrap16(vhi)

        # colidx [NSB, 128, NBLK] (block-col major: 4*lob_s lo | 4*hib hi)
        cx = np.full((NSB, 128, cfg.nblk), DEAD, dtype=np.float16)
        sn = np.zeros((NSB, 128, cfg.nblk), dtype=np.int64)
        dn = np.zeros((NSB, 128, cfg.nblk), dtype=np.int64)
        for sb in range(NSB):
            lob_s = cfg.lob_sb[sb]
            nl4 = 4 * lob_s
            nh4 = 4 * hib
            cx[sb, :, :nl4] = clo[sb, :, :lob_s].reshape(nl4, BLK).T
            cx[sb, :, nl4:nl4 + nh4] = chi[sb].reshape(nh4, BLK).T
            sn[sb, :, :nl4] = snlo[sb, :, :lob_s].reshape(nl4, BLK).T
            sn[sb, :, nl4:nl4 + nh4] = snhi[sb].reshape(nh4, BLK).T
            dn[sb, :, :nl4] = dnlo[sb, :, :lob_s].reshape(nl4, BLK).T
            dn[sb, :, nl4:nl4 + nh4] = dnhi[sb].reshape(nh4, BLK).T
        per_core.append(dict(sidx=sidx, cx=cx, sn=sn, dn=dn))
    return cfg, per_core, new_row


# -------------------------------------------------------------- device side

def build_nc1(n_cores=N_CORES):
    """Launch 1: asd [104, 512] = per-node (a_src[4], a_dst[4]) for the
    core's 6250-node slab (padded to 6272), from xT fp16 and wad8."""
    nc = bacc.Bacc("TRN2", target_bir_lowering=False, debug=False,
                   num_devices=n_cores)
    xs_t = nc.dram_tensor("xT_slab", [128, 6272], F16, kind="ExternalInput")
    wad_t = nc.dram_tensor("wad8", [IN_DIM, 2 * HEADS], F16,
                           kind="ExternalInput")
    out_t = nc.dram_tensor("adstv", [2 * HEADS, 6272], F32,
                           kind="ExternalOutput")
    with tile.TileContext(nc) as tc:
        with (tc.tile_pool(name="c", bufs=1) as cp,
              tc.tile_pool(name="p", bufs=2, space="PSUM") as pp):
            wad_c = cp.tile([IN_DIM, 2 * HEADS], F16)
            nc.sync.dma_start(wad_c[:], wad_t.ap())
            xc = cp.tile([128, 6272], F16)
            for k in range(13):
                n0 = 512 * k
                nk = min(512, 6272 - n0)
                nc.sync.dma_start(xc[:, n0:n0 + nk], xs_t.ap()[:, n0:n0 + nk])
            acc = cp.tile([2 * HEADS, 6272], F32)
            for k in range(13):
                n0 = 512 * k
                nk = min(512, 6272 - n0)
                ps = pp.tile([2 * HEADS, 512], F32, tag="ps")
                nc.tensor.matmul(ps[:, :nk], lhsT=wad_c[:],
                                 rhs=xc[:, n0:n0 + nk], start=True, stop=True)
                if k % 2 == 0:
                    nc.scalar.copy(acc[:, n0:n0 + nk], ps[:, :nk])
                else:
                    nc.vector.tensor_copy(acc[:, n0:n0 + nk], ps[:, :nk])
            nc.sync.dma_start(out_t.ap(), acc[:])
    nc.compile()
    return nc


def build_nc2(cfg: Cfg):
    nc = bacc.Bacc("TRN2", target_bir_lowering=False, debug=False,
                   num_devices=cfg.n_cores)
    lob, hib, nblk = cfg.lob, cfg.hib, cfg.nblk
    nl4 = 4 * lob
    NLO2, NHI2 = 2 * cfg.nlo, 2 * cfg.nhi

    xtab_t = nc.dram_tensor("xtab", [NPAD, IN_DIM], F16, kind="ExternalInput")
    w_t = nc.dram_tensor("w_pd", [IN_DIM, HEADS * OUT_DIM], F16,
                         kind="ExternalInput")
    idp_t = nc.dram_tensor("ident_pd", [128, 128], F16, kind="ExternalInput")
    iota_t = nc.dram_tensor("iota_pd", [128, WIN, nblk], F16,
                            kind="ExternalInput")
    biasb_t = nc.dram_tensor("biasb", [128, HEADS * OUT_DIM], F32,
                             kind="ExternalInput")
    sidx_t = nc.dram_tensor("sidx", [NPAIR, 128, (NLO2 + NHI2) // 16], I16,
                            kind="ExternalInput")
    casd_t = nc.dram_tensor("casd", [NPAIR, 128, 2, 1 + 2 * HEADS, nblk],
                            F16, kind="ExternalInput")
    out_t = nc.dram_tensor("out", [WPC * WIN, HEADS * OUT_DIM], F16,
                           kind="ExternalOutput")

    with tile.TileContext(nc) as tc:
        with tc.tile_pool(name="consts", bufs=1) as cpool:
            w_c = cpool.tile([IN_DIM, HEADS * OUT_DIM], F16)
            idp_c = cpool.tile([128, 128], F16)
            iota_c = cpool.tile([128, WIN, nblk], F16)
            biasb_c = cpool.tile([128, HEADS * OUT_DIM], F32)

            def load_consts():
                nc.sync.dma_start(w_c[:], w_t.ap())
                nc.sync.dma_start(idp_c[:], idp_t.ap())
                nc.sync.dma_start(iota_c[:].rearrange("p c b -> p (c b)"),
                                  iota_t.ap().rearrange("p c b -> p (c b)"))
                nc.sync.dma_start(biasb_c[:], biasb_t.ap())

            with (tc.tile_pool(name="st", bufs=4) as stp,
                  tc.tile_pool(name="g", bufs=4) as gp,
                  tc.tile_pool(name="al", bufs=4) as ap,
                  tc.tile_pool(name="m", bufs=4) as mp,
                  tc.tile_pool(name="z", bufs=4) as zp,
                  tc.tile_pool(name="ups", bufs=2, space="PSUM") as up,
                  tc.tile_pool(name="dps", bufs=2, space="PSUM") as dp,
                  tc.tile_pool(name="tps", bufs=2, space="PSUM") as tp,
                  tc.tile_pool(name="ops", bufs=2, space="PSUM") as op):
                for pair in range(NPAIR):
                    sidx = stp.tile([128, (NLO2 + NHI2) // 16], I16, tag="si")
                    nc.sync.dma_start(sidx[:], sidx_t.ap()[pair])
                    if pair == 0:
                        load_consts()
                    glo = gp.tile([128, NLO2 // BLK, IN_DIM], F16, tag="glo")
                    ghi = gp.tile([128, NHI2 // BLK, IN_DIM], F16, tag="ghi")
                    lob_p = cfg.lob_sb[2 * pair]
                    assert lob_p == cfg.lob_sb[2 * pair + 1]
                    nlo2 = 2 * 4 * lob_p * BLK
                    if pair < NPAIR - 1:
                        nc.gpsimd.dma_gather(glo[:, :nlo2 // BLK, :],
                                             xtab_t.ap()[0:SPLIT, :],
                                             sidx[:, :nlo2 // 16], nlo2, nlo2,
                                             IN_DIM, single_packet=False)
                        nc.gpsimd.dma_gather(ghi[:], xtab_t.ap()[SPLIT:NPAD, :],
                                             sidx[:, nlo2 // 16:
                                                  (nlo2 + NHI2) // 16],
                                             NHI2, NHI2,
                                             IN_DIM, single_packet=False)
                    else:
                        # split the tail pair per-superblock so the last
                        # compute chain overlaps the final gathers
                        nlo16, nhi16 = nlo2 // 32, NHI2 // 32
                        for s2 in range(2):
                            nc.gpsimd.dma_gather(
                                glo[:, s2 * (nlo2 // 256):nlo2 // BLK, :]
                                if s2 else glo[:, :nlo2 // 256, :],
                                xtab_t.ap()[0:SPLIT, :],
                                sidx[:, s2 * nlo16:(s2 + 1) * nlo16],
                                nlo2 // 2, nlo2 // 2, IN_DIM,
                                single_packet=False)
                            nc.gpsimd.dma_gather(
                                ghi[:, s2 * (NHI2 // 256):, :]
                                if s2 else ghi[:, :NHI2 // 256, :],
                                xtab_t.ap()[SPLIT:NPAD, :],
                                sidx[:, nlo2 // 16 + s2 * nhi16:
                                     nlo2 // 16 + (s2 + 1) * nhi16],
                                NHI2 // 2, NHI2 // 2, IN_DIM,
                                single_packet=False)
                    casd = stp.tile([128, 2, 1 + 2 * HEADS, nblk], F16,
                                    tag="casd")
                    nc.sync.dma_start(
                        casd[:].rearrange("p s e b -> p (s e b)"),
                        casd_t.ap()[pair].rearrange("p s e b -> p (s e b)"))
                    otp = zp.tile([128, 2, 128], F16, tag="otp")
                    nbu = 4 * (lob_p + hib)     # used block columns
                    for s in range(2):
                        sb = 2 * pair + s
                        cx = casd[:, s, 0, 0:nbu]
                        asd = casd[:, s, 1:1 + 2 * HEADS, 0:nbu]

                        asum = ap.tile([128, HEADS, nblk], F16, tag="asum")
                        nc.vector.tensor_tensor(out=asum[:, :, 0:nbu],
                                                in0=asd[:, 0:HEADS, :],
                                                in1=asd[:, HEADS:2 * HEADS, :],
                                                op=mybir.AluOpType.add)
                        alr = ap.tile([128, HEADS, nblk], F16, tag="alr")
                        nc.scalar.activation(alr[:, :, 0:nbu],
                                             asum[:, :, 0:nbu],
                                             mybir.ActivationFunctionType.Prelu,
                                             alpha=LEAKY_SLOPE)
                        expa = ap.tile([128, HEADS, nblk], F16, tag="expa")
                        nc.scalar.activation(expa[:, :, 0:nbu],
                                             alr[:, :, 0:nbu],
                                             mybir.ActivationFunctionType.Exp)

                        oneh = mp.tile([128, WIN, nblk], F16, tag="oneh")
                        nc.vector.tensor_tensor(
                            out=oneh[:, :, 0:nbu],
                            in0=iota_c[:, :, 0:nbu],
                            in1=cx.unsqueeze(1)
                                .to_broadcast([128, WIN, nbu]),
                            op=mybir.AluOpType.is_equal)
                        mw = mp.tile([128, HEADS, WIN, nblk], F16, tag="mw")
                        nc.vector.tensor_tensor(
                            out=mw[:, :, :, 0:nbu],
                            in0=oneh[:, :, 0:nbu].unsqueeze(1)
                                .to_broadcast([128, HEADS, WIN, nbu]),
                            in1=expa[:, :, 0:nbu].unsqueeze(2)
                                .to_broadcast([128, HEADS, WIN, nbu]),
                            op=mybir.AluOpType.mult)

                        zsb = zp.tile([128, 128], F32, tag="zsb")
                        dsb = zp.tile([128, HEADS], F32, tag="dsb")
                        nl4p = 4 * lob_p
                        for g in range(4):
                            U = up.tile([128, 128], F32, tag="U")
                            dn = dp.tile([WIN, HEADS], F32, tag="dn")
                            cols = ([g * lob_p + j for j in range(lob_p)]
                                    + [nl4p + g * hib + j for j in range(hib)])
                            for j, col in enumerate(cols):
                                if col < nl4p:
                                    rows = glo[:, s * nl4p + col, :]
                                else:
                                    rows = ghi[:, s * 4 * hib + (col - nl4p), :]
                                nc.tensor.matmul(U[:], lhsT=mw[:, :, :, col],
                                                 rhs=rows, start=(j == 0),
                                                 stop=(j == len(cols) - 1))
                            for j, col in enumerate(cols):
                                nc.tensor.matmul(dn[:], lhsT=oneh[:, :, col],
                                                 rhs=expa[:, :, col],
                                                 start=(j == 0),
                                                 stop=(j == len(cols) - 1))
                            usb = zp.tile([128, 128], F16, tag="usb")
                            nc.scalar.copy(usb[:], U[:])
                            ut_ps = tp.tile([128, 128], F16, tag="utp")
                            nc.tensor.transpose(out=ut_ps[:], in_=usb[:],
                                                identity=idp_c[:])
                            ut = zp.tile([128, 128], F16, tag="ut")
                            nc.vector.tensor_copy(ut[:], ut_ps[:])
                            o2 = op.tile([WIN, 128], F32, tag="o2")
                            for h in range(HEADS):
                                sl = slice(h * OUT_DIM, (h + 1) * OUT_DIM)
                                nc.tensor.matmul(o2[:, sl], lhsT=ut[:, sl],
                                                 rhs=w_c[:, sl],
                                                 start=True, stop=True)
                            nc.scalar.copy(zsb[g * WIN:(g + 1) * WIN, :],
                                           o2[:])
                            nc.scalar.copy(dsb[g * WIN:(g + 1) * WIN, :],
                                           dn[:])

                        rec = zp.tile([128, HEADS], F32, tag="rec")
                        nc.vector.reciprocal(rec[:], dsb[:])
                        zt = zp.tile([128, HEADS, OUT_DIM], F16, tag="zt")
                        nc.vector.tensor_tensor(
                            out=zt[:],
                            in0=zsb[:].rearrange("p (h d) -> p h d", h=HEADS),
                            in1=rec[:].unsqueeze(2)
                                .to_broadcast([128, HEADS, OUT_DIM]),
                            op=mybir.AluOpType.mult)
                        ztf = zt[:].rearrange("p h d -> p (h d)")
                        if cfg.bias_nonzero:
                            zt2 = zp.tile([128, 128], F16, tag="zt2")
                            nc.vector.tensor_tensor(out=zt2[:], in0=ztf,
                                                    in1=biasb_c[:],
                                                    op=mybir.AluOpType.add)
                            ztf = zt2[:]
                        # sigmoid via exp (stays in the exp act-table set):
                        # sg = 1/(1+exp(-z))
                        en = zp.tile([128, 128], F16, tag="en")
                        nc.scalar.activation(
                            en[:], ztf, mybir.ActivationFunctionType.Exp,
                            scale=-1.0)
                        den = zp.tile([128, 128], F32, tag="den")
                        nc.vector.tensor_scalar(den[:], en[:], 1.0, None,
                                                mybir.AluOpType.add)
                        sg = zp.tile([128, 128], F32, tag="sg")
                        nc.vector.reciprocal(sg[:], den[:])
                        mix = zp.tile([128, 128], F16, tag="mix")
                        nc.vector.tensor_scalar(mix[:], sg[:], CMIX - BETA,
                                                BETA, mybir.AluOpType.mult,
                                                mybir.AluOpType.add)
                        nc.vector.tensor_tensor(out=otp[:, s, :], in0=ztf,
                                                in1=mix[:],
                                                op=mybir.AluOpType.mult)
                        if pair == NPAIR - 1:
                            nc.sync.dma_start(
                                out_t.ap()[256 * pair:256 * pair + 256, :]
                                    .rearrange("(r s) f -> r s f", s=2)
                                    [:, s, :],
                                otp[:, s, :])
                    if pair < NPAIR - 1:
                        nc.sync.dma_start(
                            out_t.ap()[256 * pair:256 * pair + 256, :]
                                .rearrange("(r s) f -> r (s f)", s=2),
                            otp[:].rearrange("p s f -> p (s f)"))
    nc.compile()
    return nc


# ---------------------------------------------------------------- the API

def run(x, edge_index, W, att_src, att_dst, bias, n_cores=N_CORES,
        trace=False, trace_dir=None):
    x = np.asarray(x, dtype=np.float32)
    W32 = np.asarray(W, dtype=np.float32)
    att_src = np.asarray(att_src, dtype=np.float32)
    att_dst = np.asarray(att_dst, dtype=np.float32)
    bias = np.asarray(bias, dtype=np.float32)
    H, D = att_src.shape

    cfg, per_core, new_row = preprocess(edge_index, n_cores)
    cfg.bias_nonzero = bool(np.any(bias))

    # host-side param-only math + layout casts
    as4 = np.zeros((H * D, 2 * H), dtype=np.float32)
    for h in range(H):
        as4[h * D:(h + 1) * D, h] = att_src[h]
        as4[h * D:(h + 1) * D, H + h] = att_dst[h]
    wad8 = (W32 @ as4).astype(np.float16)
    xtab = np.zeros((NPAD, IN_DIM), dtype=np.float16)
    xtab[:N_NODES] = x.astype(np.float16)
    xT = np.ascontiguousarray(xtab.T)                  # [128, NPAD] fp16
    ident = np.eye(128, dtype=np.float16)
    # iota2[p, c, b] = c  (pre-expanded so the one-hot TT has packed operands)
    iota = np.broadcast_to(
        np.arange(WIN, dtype=np.float16)[None, :, None],
        (128, WIN, cfg.nblk)).copy()
    biasb = np.tile(bias, (128, 1)).astype(np.float32)

    tkw = {}
    tmp1 = tmp2 = None
    if trace:
        tkw = dict(trace=True, trace_cores=list(range(n_cores)))
        if trace_dir:
            tmp1 = os.path.join(trace_dir, "l1")
            tmp2 = os.path.join(trace_dir, "l2")
            os.makedirs(tmp1, exist_ok=True)
            os.makedirs(tmp2, exist_ok=True)

    # launch 1: per-node a_src/a_dst
    npc0 = N_NODES // n_cores
    nc1 = build_nc1(n_cores)
    in_maps1 = []
    for c in range(n_cores):
        slab = np.zeros((128, 6272), dtype=np.float16)
        slab[:, :npc0] = xT[:, c * npc0:(c + 1) * npc0]
        in_maps1.append(dict(xT_slab=slab, wad8=wad8))
    res1 = run_bass_kernel_spmd(nc1, in_maps1, core_ids=list(range(n_cores)),
                                tmpdir=tmp1, **tkw)
    asd8 = np.concatenate(
        [res1.results[c]["adstv"].T[:npc0] for c in range(n_cores)], axis=0)

    # host expansion of per-edge a_src/a_dst (indexing only)
    nc2 = build_nc2(cfg)
    in_maps = []
    for c in range(n_cores):
        pc = per_core[c]
        asd_pe = np.concatenate(
            [asd8[pc["sn"], 0:H], asd8[pc["dn"], H:2 * H]],
            axis=-1).transpose(0, 1, 3, 2)         # [NSB, 128, 8, nblk]
        casd = np.empty((NPAIR, 128, 2, 1 + 2 * H, cfg.nblk),
                        dtype=np.float16)
        casd[:, :, :, 0, :] = pc["cx"].reshape(NPAIR, 2, 128,
                                               cfg.nblk).transpose(0, 2, 1, 3)
        casd[:, :, :, 1:, :] = asd_pe.reshape(
            NPAIR, 2, 128, 2 * H, cfg.nblk).transpose(0, 2, 1, 3, 4)
        in_maps.append(dict(xtab=xtab, w_pd=W32.astype(np.float16),
                            ident_pd=ident, iota_pd=iota, biasb=biasb,
                            sidx=pc["sidx"], casd=casd))
    res = run_bass_kernel_spmd(nc2, in_maps, core_ids=list(range(n_cores)),
                               tmpdir=tmp2, **tkw)
    allout = np.concatenate([res.results[c]["out"] for c in range(n_cores)],
                            axis=0)                    # [51200, 128] fp16
    out = allout[new_row].astype(np.float32)
    parts = dict(nc1=nc1, in_maps1=in_maps1, nc2=nc2, in_maps2=in_maps,
                 res1=res1, res2=res, n_cores=n_cores, cfg=cfg)
    return out, parts


def kernel(**inputs) -> np.ndarray:
    out, _ = run(inputs["x"], inputs["edge_index"], inputs["W"],
                 inputs["att_src"], inputs["att_dst"], inputs["bias"])
    return out
